# revision 6
# baseline (speedup 1.0000x reference)
"""Sliding-window attention + residual + LayerNorm on 8 Trainium2 NeuronCores.

Problem (hardcoded): B=1, S=4096, HID=1024, NH=16, HD=64, WIN=256.
    q,k,v = X@W* + b*  (per-head HD=64)
    scores = q k^T / 8, sliding-window mask (j in [i-128, i+128)), softmax
    out = LayerNorm(X + probs@v) * gamma + beta

Sharding: sequence-parallel. Core c owns query rows [c*512, c*512+512) and
receives X rows [c*512-128, c*512+640) (zero-padded at the sequence edges) so
all K/V it attends to are computed locally (halo recompute, no collectives).

Per-core kernel layout (all SBUF tiles are [128 partitions, ...]):
  - X is PE-transposed once into XT [h, s] (fp32r) and reused by all three
    projections.
  - QT/KT are produced transposed ([d, s], head-major: head h = d-chunk h//2,
    partition half h%2) directly from the projection matmul; V stays natural
    [s, d] with a ones-column appended per head (V_aug [s, 65]) so the PV
    matmul also produces the softmax denominator Z for free.
  - scores are computed TRANSPOSED (scoresT[j, i] = kT.T @ qT) so no
    transpose of probabilities is ever needed; softmax skips max-subtraction
    (|scores| <= ~8, exp cannot overflow; masked entries are multiplied by 0
    after exp which matches the reference's exp(-10000-max) underflow).
  - matmuls run in float32r (TF32-like, ~1.5e-4 rel err, 4x faster than fp32).
"""

import numpy as np

import concourse.bass as bass
import concourse.tile as tile
from concourse import bacc, mybir
from concourse import bass_utils
from concourse.masks import make_identity

F32 = mybir.dt.float32
F32R = mybir.dt.float32r
AFT = mybir.ActivationFunctionType

S, HID, NH, HD = 4096, 1024, 16, 64
WIN = 256
EPS = 1e-12
NCORES = 8
SLOC = S // NCORES          # 512 own rows per core
HALO = WIN // 2             # 128
KLOC = SLOC + 2 * HALO      # 768 local K/V rows
NB = SLOC // 128            # 4 query blocks per core
NKC = KLOC // 128           # 6 local K chunks
HCH = HID // 128            # 8 hidden chunks
SCALE = 1.0 / np.sqrt(HD)


def _emit(nc, tc, ctx, d):
    """Emit the per-core kernel into TileContext tc. d: dict of DRAM APs."""
    const = ctx.enter_context(tc.tile_pool(name="const", bufs=1))
    big = ctx.enter_context(tc.tile_pool(name="big", bufs=1))
    wstream = ctx.enter_context(tc.tile_pool(name="wstream", bufs=8))
    wres = ctx.enter_context(tc.tile_pool(name="wres", bufs=1))
    expm_p = ctx.enter_context(tc.tile_pool(name="expm", bufs=6))
    temps = ctx.enter_context(tc.tile_pool(name="temps", bufs=4))
    small = ctx.enter_context(tc.tile_pool(name="small", bufs=8))
    ctx_p = ctx.enter_context(tc.tile_pool(name="ctxp", bufs=2))
    ps = ctx.enter_context(tc.tile_pool(name="ps", bufs=4, space="PSUM"))

    def ps_tile(shape):
        return ps.tile(shape, F32, tag="ps", name="ps")

    # ---- constants ----
    ident = const.tile([128, 128], F32)
    make_identity(nc, ident)
    bqs_sb = const.tile([128, HCH], F32)   # 0.125*bq, [d%128, d//128]
    bk_sb = const.tile([128, HCH], F32)
    nc.sync.dma_start(out=bqs_sb, in_=d["bqs"].rearrange("(c p) -> p c", p=128))
    nc.sync.dma_start(out=bk_sb, in_=d["bk"].rearrange("(c p) -> p c", p=128))

    def bcast(src_ap):
        t = const.tile([128, HID], F32)
        nc.sync.dma_start(
            out=t,
            in_=bass.AP(tensor=src_ap.tensor, offset=src_ap.offset,
                        ap=[[0, 128]] + src_ap.ap),
        )
        return t

    bv_b = bcast(d["bv"])
    gamma_b = bcast(d["gamma"])
    beta_b = bcast(d["beta"])
    eps_t = const.tile([128, 1], F32)
    nc.vector.memset(eps_t, EPS)
    # masks, transposed: [jc, t, side, i]
    maskt_sb = const.tile([128, NB, 2, 128], F32)
    nc.sync.dma_start(
        out=maskt_sb,
        in_=d["maskt"].rearrange("t s j i -> j t s i"),
    )

    # ---- stage A: load X, build XT ----
    x_all = big.tile([128, NKC, HID], F32)
    with nc.named_scope("load_x"):
        for sc in range(NKC):
            nc.sync.dma_start(out=x_all[:, sc, :],
                              in_=d["xh"][sc * 128:(sc + 1) * 128, :])
    xt_all = big.tile([128, HCH, KLOC], F32R)
    with nc.named_scope("transpose_x"):
        for sc in range(NKC):
            for hc in range(HCH):
                tp = ps_tile([128, 128])
                nc.tensor.transpose(tp, x_all[:, sc, hc * 128:(hc + 1) * 128],
                                    ident)
                nc.vector.tensor_copy(
                    out=xt_all[:, hc, sc * 128:(sc + 1) * 128], in_=tp)

    # ---- stage B: projections ----
    qt_all = big.tile([128, HCH, SLOC], F32R)   # [d, dc, own s]
    kt_all = big.tile([128, HCH, KLOC], F32R)   # [d, dc, local s]
    v_all = big.tile([128, NKC, NH, HD + 2], F32R)  # [s, sc, head, 64+ones+pad]

    with nc.named_scope("proj_q"):
        for dc in range(HCH):
            pq = ps_tile([128, SLOC])
            for hc in range(HCH):
                wq_t = wstream.tile([128, 128], F32R, tag="wq_t")
                nc.sync.dma_start(
                    out=wq_t,
                    in_=d["wq"][hc * 128:(hc + 1) * 128, dc * 128:(dc + 1) * 128])
                nc.tensor.matmul(pq, lhsT=wq_t[:],
                                 rhs=xt_all[:, hc, HALO:HALO + SLOC],
                                 start=(hc == 0), stop=(hc == HCH - 1))
            nc.vector.tensor_scalar(out=qt_all[:, dc, :], in0=pq,
                                    scalar1=SCALE, scalar2=bqs_sb[:, dc:dc + 1],
                                    op0=mybir.AluOpType.mult,
                                    op1=mybir.AluOpType.add)

    with nc.named_scope("proj_k"):
        for dc in range(HCH):
            pk = ps_tile([128, KLOC])
            for hc in range(HCH):
                wk_t = wstream.tile([128, 128], F32R, tag="wk_t")
                nc.sync.dma_start(
                    out=wk_t,
                    in_=d["wk"][hc * 128:(hc + 1) * 128, dc * 128:(dc + 1) * 128])
                nc.tensor.matmul(pk[:, 0:512], lhsT=wk_t[:],
                                 rhs=xt_all[:, hc, 0:512],
                                 start=(hc == 0), stop=(hc == HCH - 1))
                nc.tensor.matmul(pk[:, 512:KLOC], lhsT=wk_t[:],
                                 rhs=xt_all[:, hc, 512:KLOC],
                                 start=(hc == 0), stop=(hc == HCH - 1))
            nc.vector.tensor_scalar_add(out=kt_all[:, dc, :], in0=pk,
                                        scalar1=bk_sb[:, dc:dc + 1])

    with nc.named_scope("proj_v"):
        for nh_ in range(2):
            wv_t = wres.tile([128, HCH, 512], F32R, tag="wv_t")
            nc.sync.dma_start(
                out=wv_t,
                in_=d["wv"].rearrange("(hc p) d -> p hc d", p=128)
                [:, :, nh_ * 512:(nh_ + 1) * 512])
            for sc in range(NKC):
                pv = ps_tile([128, 512])
                for hc in range(HCH):
                    nc.tensor.matmul(pv, lhsT=xt_all[:, hc, sc * 128:(sc + 1) * 128],
                                     rhs=wv_t[:, hc, :],
                                     start=(hc == 0), stop=(hc == HCH - 1))
                nc.vector.tensor_add(
                    out=v_all[:, sc, nh_ * 8:(nh_ + 1) * 8, 0:HD],
                    in0=pv[:].rearrange("p (h e) -> p h e", e=HD),
                    in1=bv_b[:, nh_ * 512:(nh_ + 1) * 512]
                    .rearrange("p (h e) -> p h e", e=HD))
        ones_f = const.tile([128, 2], F32)
        nc.vector.memset(ones_f[:, 0:1], 1.0)
        nc.vector.memset(ones_f[:, 1:2], 0.0)
        ones_r = const.tile([128, 2], F32R)
        nc.vector.tensor_copy(out=ones_r, in_=ones_f)
        for sc in range(NKC):
            nc.vector.tensor_copy(
                out=v_all[:, sc, :, HD:HD + 2],
                in_=ones_r.unsqueeze(1).to_broadcast([128, NH, 2]))

    # ---- stage C: attention ----
    ctx_sb = {}
    for p in range(2):                      # pairs of query blocks
        expm = {}
        for t in (2 * p, 2 * p + 1):
            ctx_sb[t] = ctx_p.tile([128, NH, HD], F32, tag="ctx_sb", name="ctx_sb")
        cps = {}
        for h in range(NH):
            dc, ph = h // 2, (h % 2) * 64
            pscore = ps_tile([128, 4, 256])
            with nc.named_scope("scores"):
                for m in range(4):
                    nc.tensor.matmul(
                        pscore[:, m, :],
                        lhsT=kt_all[ph:ph + 64, dc, (2 * p + m) * 128:(2 * p + m + 1) * 128],
                        rhs=qt_all[ph:ph + 64, dc, p * 256:(p + 1) * 256],
                        start=True, stop=True)
            for q in range(2):
                t = 2 * p + q
                em = expm_p.tile([128, 3, 128], F32R, tag="expm")
                with nc.named_scope("softmax"):
                    nc.scalar.activation(
                        out=em, in_=pscore[:, q:q + 3, q * 128:(q + 1) * 128],
                        func=AFT.Exp)
                    nc.vector.tensor_mul(
                        out=em[:, 0::2, :], in0=em[:, 0::2, :],
                        in1=maskt_sb[:, t, :, :])
                g, hi = h // 4, h % 4
                if hi == 0:
                    cps[(q, g)] = ps.tile([128, 4, HD + 2], F32, tag="ps", name="cps")
                with nc.named_scope("pv"):
                    for c in range(3):
                        nc.tensor.matmul(
                            cps[(q, g)][:, hi, :],
                            lhsT=em[:, c, :],
                            rhs=v_all[:, t + c, h, :],
                            start=(c == 0), stop=(c == 2))
                if hi == 3:
                    with nc.named_scope("ctx_scale"):
                        zv = small.tile([128, 4], F32, tag="zv")
                        nc.vector.tensor_copy(out=zv, in_=cps[(q, g)][:, :, HD])
                        nc.vector.reciprocal(out=zv, in_=zv)
                        nc.vector.tensor_mul(
                            out=ctx_sb[t][:, 4 * g:4 * g + 4, :],
                            in0=cps[(q, g)][:, :, 0:HD],
                            in1=zv.unsqueeze(2).to_broadcast([128, 4, HD]))

        # ---- stage D: residual + layernorm for the two finished blocks ----
        for q in range(2):
            t = 2 * p + q
            with nc.named_scope("layernorm"):
                xs = temps.tile([128, HID], F32, tag="xs")
                nc.vector.tensor_add(
                    out=xs, in0=x_all[:, t + 1, :],
                    in1=ctx_sb[t][:].rearrange("p h e -> p (h e)"))
                stats = small.tile([128, 2, 6], F32, tag="stats")
                for sg in range(2):
                    nc.vector.bn_stats(out=stats[:, sg, :],
                                       in_=xs[:, sg * 512:(sg + 1) * 512])
                mv = small.tile([128, 2], F32, tag="mv")
                nc.vector.bn_aggr(out=mv, in_=stats)
                rstd = small.tile([128, 1], F32, tag="rstd")
                nc.scalar.activation(out=rstd, in_=mv[:, 1:2], func=AFT.Sqrt,
                                     bias=eps_t)
                nc.vector.reciprocal(out=rstd, in_=rstd)
                xn = temps.tile([128, HID], F32, tag="xn")
                nc.vector.tensor_scalar(out=xn, in0=xs, scalar1=mv[:, 0:1],
                                        scalar2=rstd,
                                        op0=mybir.AluOpType.subtract,
                                        op1=mybir.AluOpType.mult)
                nc.vector.tensor_mul(out=xn, in0=xn, in1=gamma_b)
                nc.vector.tensor_add(out=xn, in0=xn, in1=beta_b)
                nc.sync.dma_start(out=d["out"][t * 128:(t + 1) * 128, :], in_=xn)


def build_module():
    nc = bacc.Bacc("TRN2", target_bir_lowering=False, debug=False,
                   num_devices=NCORES)
    d = {
        "xh": nc.dram_tensor("xh", [KLOC, HID], F32, kind="ExternalInput").ap(),
        "wq": nc.dram_tensor("wq", [HID, HID], F32R, kind="ExternalInput").ap(),
        "wk": nc.dram_tensor("wk", [HID, HID], F32R, kind="ExternalInput").ap(),
        "wv": nc.dram_tensor("wv", [HID, HID], F32R, kind="ExternalInput").ap(),
        "bqs": nc.dram_tensor("bqs", [HID], F32, kind="ExternalInput").ap(),
        "bk": nc.dram_tensor("bk", [HID], F32, kind="ExternalInput").ap(),
        "bv": nc.dram_tensor("bv", [HID], F32, kind="ExternalInput").ap(),
        "gamma": nc.dram_tensor("gamma", [HID], F32, kind="ExternalInput").ap(),
        "beta": nc.dram_tensor("beta", [HID], F32, kind="ExternalInput").ap(),
        "maskt": nc.dram_tensor("maskt", [NB, 2, 128, 128], F32,
                                kind="ExternalInput").ap(),
        "out": nc.dram_tensor("out", [SLOC, HID], F32, kind="ExternalOutput").ap(),
    }
    from contextlib import ExitStack
    with tile.TileContext(nc) as tc:
        with ExitStack() as ctx:
            _emit(nc, tc, ctx, d)
    nc.compile()
    return nc


def _make_masks():
    """maskt[core][t, side, jc, i]: 1.0 keep / 0.0 drop, scoresT orientation."""
    jc = np.arange(128)[:, None]
    i = np.arange(128)[None, :]
    band = [jc >= i, jc < i]              # side 0: chunk m=0; side 1: chunk m=2
    masks = np.zeros((NCORES, NB, 2, 128, 128), np.float32)
    for c in range(NCORES):
        for t in range(NB):
            k0 = c * SLOC + t * 128 - HALO     # global j of local chunk col 0
            for side, m in ((0, 0), (1, 2)):
                jg = k0 + m * 128 + jc
                valid = (jg >= 0) & (jg < S)
                masks[c, t, side] = (band[side] & valid).astype(np.float32)
    return masks


_STATE = {}


def kernel(**inputs):
    hs = np.asarray(inputs["hidden_states"], np.float32).reshape(S, HID)
    wq = np.ascontiguousarray(np.asarray(inputs["Wq"], np.float32))
    wk = np.ascontiguousarray(np.asarray(inputs["Wk"], np.float32))
    wv = np.ascontiguousarray(np.asarray(inputs["Wv"], np.float32))
    bq = np.asarray(inputs["bq"], np.float32)
    bk = np.asarray(inputs["bk"], np.float32)
    bv = np.asarray(inputs["bv"], np.float32)
    gamma = np.asarray(inputs["gamma"], np.float32)
    beta = np.asarray(inputs["beta"], np.float32)

    if "nc" not in _STATE:
        _STATE["nc"] = build_module()
        _STATE["masks"] = _make_masks()
    nc = _STATE["nc"]
    masks = _STATE["masks"]

    xpad = np.zeros((S + 2 * HALO, HID), np.float32)
    xpad[HALO:HALO + S] = hs
    common = {"wq": wq, "wk": wk, "wv": wv, "bqs": (SCALE * bq).astype(np.float32),
              "bk": bk, "bv": bv, "gamma": gamma, "beta": beta}
    in_maps = [
        {**common, "xh": np.ascontiguousarray(xpad[c * SLOC:c * SLOC + KLOC]),
         "maskt": np.ascontiguousarray(masks[c])}
        for c in range(NCORES)
    ]
    res = bass_utils.run_bass_kernel_spmd(nc, in_maps,
                                          core_ids=list(range(NCORES)),
                                          **_STATE.get("run_kwargs", {}))
    _STATE["last_result"] = res
    out = np.concatenate([res.results[c]["out"] for c in range(NCORES)], axis=0)
    return out.reshape(1, S, HID)


# revision 7
# speedup vs baseline: 1.1235x; 1.1235x over previous
"""Sliding-window attention + residual + LayerNorm on 8 Trainium2 NeuronCores.

Problem (hardcoded): B=1, S=4096, HID=1024, NH=16, HD=64, WIN=256.
    q,k,v = X@W* + b*  (per-head HD=64)
    scores = q k^T / 8, sliding-window mask (j in [i-128, i+128)), softmax
    out = LayerNorm(X + probs@v) * gamma + beta

Sharding: sequence-parallel. Core c owns query rows [c*512, c*512+512) and
receives X rows [c*512-128, c*512+640) (zero-padded at the sequence edges) so
all K/V it attends to are computed locally (halo recompute, no collectives).

Per-core kernel layout (all SBUF tiles are [128 partitions, ...]):
  - X is PE-transposed once into XT [h, s] (fp32r) and reused by all three
    projections.
  - QT/KT are produced transposed ([d, s], head-major: head h = d-chunk h//2,
    partition half h%2) directly from the projection matmul; V stays natural
    [s, d] with a ones-column appended per head (V_aug [s, 65]) so the PV
    matmul also produces the softmax denominator Z for free.
  - scores are computed TRANSPOSED (scoresT[j, i] = kT.T @ qT) so no
    transpose of probabilities is ever needed; softmax skips max-subtraction
    (|scores| <= ~8, exp cannot overflow; masked entries are multiplied by 0
    after exp which matches the reference's exp(-10000-max) underflow).
  - matmuls run in float32r (TF32-like, ~1.5e-4 rel err, 4x faster than fp32).
"""

import numpy as np

import concourse.bass as bass
import concourse.tile as tile
from concourse import bacc, mybir
from concourse import bass_utils
from concourse.masks import make_identity

F32 = mybir.dt.float32
F32R = mybir.dt.float32r
F16 = mybir.dt.float16
AFT = mybir.ActivationFunctionType

S, HID, NH, HD = 4096, 1024, 16, 64
WIN = 256
EPS = 1e-12
NCORES = 8
SLOC = S // NCORES          # 512 own rows per core
HALO = WIN // 2             # 128
KLOC = SLOC + 2 * HALO      # 768 local K/V rows
NB = SLOC // 128            # 4 query blocks per core
NKC = KLOC // 128           # 6 local K chunks
HCH = HID // 128            # 8 hidden chunks
SCALE = 1.0 / np.sqrt(HD)


def _emit(nc, tc, ctx, d):
    """Emit the per-core kernel into TileContext tc. d: dict of DRAM APs."""
    const = ctx.enter_context(tc.tile_pool(name="const", bufs=1))
    big = ctx.enter_context(tc.tile_pool(name="big", bufs=1))
    wstream = ctx.enter_context(tc.tile_pool(name="wstream", bufs=12))
    wres = ctx.enter_context(tc.tile_pool(name="wres", bufs=1))
    expm_p = ctx.enter_context(tc.tile_pool(name="expm", bufs=6))
    temps = ctx.enter_context(tc.tile_pool(name="temps", bufs=4))
    small = ctx.enter_context(tc.tile_pool(name="small", bufs=8))
    ctx_p = ctx.enter_context(tc.tile_pool(name="ctxp", bufs=2))
    ps = ctx.enter_context(tc.tile_pool(name="ps", bufs=4, space="PSUM"))

    def ps_tile(shape):
        return ps.tile(shape, F32, tag="ps", name="ps")

    # ---- constants ----
    ident = const.tile([128, 128], F32)
    make_identity(nc, ident)
    bqs_sb = const.tile([128, HCH], F32)   # 0.125*bq, [d%128, d//128]
    bk_sb = const.tile([128, HCH], F32)
    nc.sync.dma_start(out=bqs_sb, in_=d["bqs"].rearrange("(c p) -> p c", p=128))
    nc.sync.dma_start(out=bk_sb, in_=d["bk"].rearrange("(c p) -> p c", p=128))

    def bcast(src_ap):
        t = const.tile([128, HID], F32)
        nc.sync.dma_start(
            out=t,
            in_=bass.AP(tensor=src_ap.tensor, offset=src_ap.offset,
                        ap=[[0, 128]] + src_ap.ap),
        )
        return t

    bv_b = bcast(d["bv"])
    gamma_b = bcast(d["gamma"])
    beta_b = bcast(d["beta"])
    eps_t = const.tile([128, 1], F32)
    nc.vector.memset(eps_t, EPS)
    # masks, transposed: [jc, t, side, i]
    maskt_sb = const.tile([128, NB, 2, 128], F16)
    nc.sync.dma_start(
        out=maskt_sb,
        in_=d["maskt"].rearrange("t s j i -> j t s i"),
    )

    # ---- stage A: load X, build XT ----
    x_all = big.tile([128, NKC, HID], F32)
    with nc.named_scope("load_x"):
        for sc in range(NKC):
            nc.sync.dma_start(out=x_all[:, sc, :],
                              in_=d["xh"][sc * 128:(sc + 1) * 128, :])
    xt_all = big.tile([128, HCH, KLOC], F32R)
    with nc.named_scope("transpose_x"):
        for sc in range(NKC):
            for hc in range(HCH):
                tp = ps_tile([128, 128])
                nc.tensor.transpose(tp, x_all[:, sc, hc * 128:(hc + 1) * 128],
                                    ident)
                nc.vector.tensor_copy(
                    out=xt_all[:, hc, sc * 128:(sc + 1) * 128], in_=tp)

    # ---- stage B: projections ----
    qt_all = big.tile([128, HCH, SLOC], F16)   # [d, dc, own s]
    kt_all = big.tile([128, HCH, KLOC], F16)   # [d, dc, local s]
    v_all = big.tile([128, NKC, NH, HD + 2], F16)  # [s, sc, head, 64+ones+pad]

    with nc.named_scope("proj_q"):
        for dc in range(HCH):
            pq = ps_tile([128, SLOC])
            for hc in range(HCH):
                wq_t = wstream.tile([128, 128], F32R, tag="wq_t")
                nc.sync.dma_start(
                    out=wq_t,
                    in_=d["wq"][hc * 128:(hc + 1) * 128, dc * 128:(dc + 1) * 128])
                nc.tensor.matmul(pq, lhsT=wq_t[:],
                                 rhs=xt_all[:, hc, HALO:HALO + SLOC],
                                 start=(hc == 0), stop=(hc == HCH - 1))
            nc.vector.tensor_scalar(out=qt_all[:, dc, :], in0=pq,
                                    scalar1=SCALE, scalar2=bqs_sb[:, dc:dc + 1],
                                    op0=mybir.AluOpType.mult,
                                    op1=mybir.AluOpType.add)

    with nc.named_scope("proj_k"):
        for dc in range(HCH):
            pk = ps_tile([128, KLOC])
            for hc in range(HCH):
                wk_t = wstream.tile([128, 128], F32R, tag="wk_t")
                nc.sync.dma_start(
                    out=wk_t,
                    in_=d["wk"][hc * 128:(hc + 1) * 128, dc * 128:(dc + 1) * 128])
                nc.tensor.matmul(pk[:, 0:512], lhsT=wk_t[:],
                                 rhs=xt_all[:, hc, 0:512],
                                 start=(hc == 0), stop=(hc == HCH - 1))
                nc.tensor.matmul(pk[:, 512:KLOC], lhsT=wk_t[:],
                                 rhs=xt_all[:, hc, 512:KLOC],
                                 start=(hc == 0), stop=(hc == HCH - 1))
            nc.vector.tensor_scalar_add(out=kt_all[:, dc, :], in0=pk,
                                        scalar1=bk_sb[:, dc:dc + 1])

    with nc.named_scope("proj_v"):
        for nh_ in range(2):
            wv_t = wres.tile([128, HCH, 512], F32R, tag="wv_t")
            nc.sync.dma_start(
                out=wv_t,
                in_=d["wv"].rearrange("(hc p) d -> p hc d", p=128)
                [:, :, nh_ * 512:(nh_ + 1) * 512])
            for sc in range(NKC):
                pv = ps_tile([128, 512])
                for hc in range(HCH):
                    nc.tensor.matmul(pv, lhsT=xt_all[:, hc, sc * 128:(sc + 1) * 128],
                                     rhs=wv_t[:, hc, :],
                                     start=(hc == 0), stop=(hc == HCH - 1))
                nc.vector.tensor_add(
                    out=v_all[:, sc, nh_ * 8:(nh_ + 1) * 8, 0:HD],
                    in0=pv[:].rearrange("p (h e) -> p h e", e=HD),
                    in1=bv_b[:, nh_ * 512:(nh_ + 1) * 512]
                    .rearrange("p (h e) -> p h e", e=HD))
        ones_f = const.tile([128, 2], F32)
        nc.vector.memset(ones_f[:, 0:1], 1.0)
        nc.vector.memset(ones_f[:, 1:2], 0.0)
        ones_r = const.tile([128, 2], F16)
        nc.vector.tensor_copy(out=ones_r, in_=ones_f)
        for sc in range(NKC):
            nc.vector.tensor_copy(
                out=v_all[:, sc, :, HD:HD + 2],
                in_=ones_r.unsqueeze(1).to_broadcast([128, NH, 2]))

    # ---- stage C: attention ----
    ctx_sb = {}
    for p in range(2):                      # pairs of query blocks
        expm = {}
        for t in (2 * p, 2 * p + 1):
            ctx_sb[t] = ctx_p.tile([128, NH, HD], F32, tag="ctx_sb", name="ctx_sb")
        cps = {}
        for h in range(NH):
            dc, ph = h // 2, (h % 2) * 64
            pscore = ps_tile([128, 4, 256])
            with nc.named_scope("scores"):
                for m in range(4):
                    nc.tensor.matmul(
                        pscore[:, m, :],
                        lhsT=kt_all[ph:ph + 64, dc, (2 * p + m) * 128:(2 * p + m + 1) * 128],
                        rhs=qt_all[ph:ph + 64, dc, p * 256:(p + 1) * 256],
                        start=True, stop=True)
            for q in range(2):
                t = 2 * p + q
                em = expm_p.tile([128, 3, 128], F16, tag="expm")
                with nc.named_scope("softmax"):
                    nc.scalar.activation(
                        out=em, in_=pscore[:, q:q + 3, q * 128:(q + 1) * 128],
                        func=AFT.Exp)
                    nc.vector.tensor_mul(
                        out=em[:, 0::2, :], in0=em[:, 0::2, :],
                        in1=maskt_sb[:, t, :, :])
                g, hi = h // 4, h % 4
                if hi == 0:
                    cps[(q, g)] = ps.tile([128, 4, HD + 2], F32, tag="ps", name="cps")
                with nc.named_scope("pv"):
                    for c in range(3):
                        nc.tensor.matmul(
                            cps[(q, g)][:, hi, :],
                            lhsT=em[:, c, :],
                            rhs=v_all[:, t + c, h, :],
                            start=(c == 0), stop=(c == 2))
                if hi == 3:
                    with nc.named_scope("ctx_scale"):
                        zv = small.tile([128, 4], F32, tag="zv")
                        nc.vector.tensor_copy(out=zv, in_=cps[(q, g)][:, :, HD])
                        nc.vector.reciprocal(out=zv, in_=zv)
                        nc.vector.tensor_mul(
                            out=ctx_sb[t][:, 4 * g:4 * g + 4, :],
                            in0=cps[(q, g)][:, :, 0:HD],
                            in1=zv.unsqueeze(2).to_broadcast([128, 4, HD]))

        # ---- stage D: residual + layernorm for the two finished blocks ----
        for q in range(2):
            t = 2 * p + q
            with nc.named_scope("layernorm"):
                xs = temps.tile([128, HID], F32, tag="xs")
                nc.vector.tensor_add(
                    out=xs, in0=x_all[:, t + 1, :],
                    in1=ctx_sb[t][:].rearrange("p h e -> p (h e)"))
                stats = small.tile([128, 2, 6], F32, tag="stats")
                for sg in range(2):
                    nc.vector.bn_stats(out=stats[:, sg, :],
                                       in_=xs[:, sg * 512:(sg + 1) * 512])
                mv = small.tile([128, 2], F32, tag="mv")
                nc.vector.bn_aggr(out=mv, in_=stats)
                rstd = small.tile([128, 1], F32, tag="rstd")
                nc.scalar.activation(out=rstd, in_=mv[:, 1:2], func=AFT.Sqrt,
                                     bias=eps_t)
                nc.vector.reciprocal(out=rstd, in_=rstd)
                xn = temps.tile([128, HID], F32, tag="xn")
                nc.vector.tensor_scalar(out=xn, in0=xs, scalar1=mv[:, 0:1],
                                        scalar2=rstd,
                                        op0=mybir.AluOpType.subtract,
                                        op1=mybir.AluOpType.mult)
                nc.vector.tensor_mul(out=xn, in0=xn, in1=gamma_b)
                nc.vector.tensor_add(out=xn, in0=xn, in1=beta_b)
                nc.sync.dma_start(out=d["out"][t * 128:(t + 1) * 128, :], in_=xn)


def build_module():
    nc = bacc.Bacc("TRN2", target_bir_lowering=False, debug=False,
                   num_devices=NCORES)
    d = {
        "xh": nc.dram_tensor("xh", [KLOC, HID], F32, kind="ExternalInput").ap(),
        "wq": nc.dram_tensor("wq", [HID, HID], F32R, kind="ExternalInput").ap(),
        "wk": nc.dram_tensor("wk", [HID, HID], F32R, kind="ExternalInput").ap(),
        "wv": nc.dram_tensor("wv", [HID, HID], F32R, kind="ExternalInput").ap(),
        "bqs": nc.dram_tensor("bqs", [HID], F32, kind="ExternalInput").ap(),
        "bk": nc.dram_tensor("bk", [HID], F32, kind="ExternalInput").ap(),
        "bv": nc.dram_tensor("bv", [HID], F32, kind="ExternalInput").ap(),
        "gamma": nc.dram_tensor("gamma", [HID], F32, kind="ExternalInput").ap(),
        "beta": nc.dram_tensor("beta", [HID], F32, kind="ExternalInput").ap(),
        "maskt": nc.dram_tensor("maskt", [NB, 2, 128, 128], mybir.dt.float16,
                                kind="ExternalInput").ap(),
        "out": nc.dram_tensor("out", [SLOC, HID], F32, kind="ExternalOutput").ap(),
    }
    from contextlib import ExitStack
    with tile.TileContext(nc) as tc:
        with ExitStack() as ctx:
            _emit(nc, tc, ctx, d)
    nc.compile()
    return nc


def _make_masks():
    """maskt[core][t, side, jc, i]: 1.0 keep / 0.0 drop, scoresT orientation."""
    jc = np.arange(128)[:, None]
    i = np.arange(128)[None, :]
    band = [jc >= i, jc < i]              # side 0: chunk m=0; side 1: chunk m=2
    masks = np.zeros((NCORES, NB, 2, 128, 128), np.float32)
    for c in range(NCORES):
        for t in range(NB):
            k0 = c * SLOC + t * 128 - HALO     # global j of local chunk col 0
            for side, m in ((0, 0), (1, 2)):
                jg = k0 + m * 128 + jc
                valid = (jg >= 0) & (jg < S)
                masks[c, t, side] = (band[side] & valid).astype(np.float32)
    return masks


_STATE = {}


def kernel(**inputs):
    hs = np.asarray(inputs["hidden_states"], np.float32).reshape(S, HID)
    wq = np.ascontiguousarray(np.asarray(inputs["Wq"], np.float32))
    wk = np.ascontiguousarray(np.asarray(inputs["Wk"], np.float32))
    wv = np.ascontiguousarray(np.asarray(inputs["Wv"], np.float32))
    bq = np.asarray(inputs["bq"], np.float32)
    bk = np.asarray(inputs["bk"], np.float32)
    bv = np.asarray(inputs["bv"], np.float32)
    gamma = np.asarray(inputs["gamma"], np.float32)
    beta = np.asarray(inputs["beta"], np.float32)

    if "nc" not in _STATE:
        _STATE["nc"] = build_module()
        _STATE["masks"] = _make_masks()
    nc = _STATE["nc"]
    masks = _STATE["masks"]

    xpad = np.zeros((S + 2 * HALO, HID), np.float32)
    xpad[HALO:HALO + S] = hs
    common = {"wq": wq, "wk": wk, "wv": wv, "bqs": (SCALE * bq).astype(np.float32),
              "bk": bk, "bv": bv, "gamma": gamma, "beta": beta}
    in_maps = [
        {**common, "xh": np.ascontiguousarray(xpad[c * SLOC:c * SLOC + KLOC]),
         "maskt": np.ascontiguousarray(masks[c].astype(np.float16))}
        for c in range(NCORES)
    ]
    res = bass_utils.run_bass_kernel_spmd(nc, in_maps,
                                          core_ids=list(range(NCORES)),
                                          **_STATE.get("run_kwargs", {}))
    _STATE["last_result"] = res
    out = np.concatenate([res.results[c]["out"] for c in range(NCORES)], axis=0)
    return out.reshape(1, S, HID)


# revision 11
# speedup vs baseline: 1.3619x; 1.2121x over previous
"""Sliding-window attention + residual + LayerNorm on 8 Trainium2 NeuronCores.

Problem (hardcoded): B=1, S=4096, HID=1024, NH=16, HD=64, WIN=256.
    q,k,v = X@W* + b*  (per-head HD=64)
    scores = q k^T / 8, sliding-window mask (j in [i-128, i+128)), softmax
    out = LayerNorm(X + probs@v) * gamma + beta

Sharding: sequence-parallel. Core c owns query rows [c*512, c*512+512) and
receives X rows [c*512-128, c*512+640) (zero-padded at the sequence edges) so
all K/V it attends to are computed locally (halo recompute, no collectives).

Per-core kernel layout (all SBUF tiles are [128 partitions, ...]):
  - X is PE-transposed once into XT [h, s] (fp32r) and reused by all three
    projections.
  - QT/KT are produced transposed ([d, s], head-major: head h = d-chunk h//2,
    partition half h%2) directly from the projection matmul; V stays natural
    [s, d] with a ones-column appended per head (V_aug [s, 65]) so the PV
    matmul also produces the softmax denominator Z for free.
  - scores are computed TRANSPOSED (scoresT[j, i] = kT.T @ qT) so no
    transpose of probabilities is ever needed; softmax skips max-subtraction
    (|scores| <= ~8, exp cannot overflow; masked entries are multiplied by 0
    after exp which matches the reference's exp(-10000-max) underflow).
  - matmuls run in float32r (TF32-like, ~1.5e-4 rel err, 4x faster than fp32).
"""

import numpy as np

import concourse.bass as bass
import concourse.tile as tile
from concourse import bacc, mybir
from concourse import bass_utils
from concourse.masks import make_identity

F32 = mybir.dt.float32
F32R = mybir.dt.float32r
F16 = mybir.dt.float16
AFT = mybir.ActivationFunctionType

S, HID, NH, HD = 4096, 1024, 16, 64
WIN = 256
EPS = 1e-12
NCORES = 8
SLOC = S // NCORES          # 512 own rows per core
HALO = WIN // 2             # 128
KLOC = SLOC + 2 * HALO      # 768 local K/V rows
NB = SLOC // 128            # 4 query blocks per core
NKC = KLOC // 128           # 6 local K chunks
HCH = HID // 128            # 8 hidden chunks
SCALE = 1.0 / np.sqrt(HD)


def _emit(nc, tc, ctx, d):
    """Emit the per-core kernel into TileContext tc. d: dict of DRAM APs."""
    const = ctx.enter_context(tc.tile_pool(name="const", bufs=1))
    big = ctx.enter_context(tc.tile_pool(name="big", bufs=1))
    wstream = ctx.enter_context(tc.tile_pool(name="wstream", bufs=3))
    wres = ctx.enter_context(tc.tile_pool(name="wres", bufs=2))
    expm_p = ctx.enter_context(tc.tile_pool(name="expm", bufs=8))
    temps = ctx.enter_context(tc.tile_pool(name="temps", bufs=4))
    small = ctx.enter_context(tc.tile_pool(name="small", bufs=8))
    ctx_p = ctx.enter_context(tc.tile_pool(name="ctxp", bufs=2))

    # ---- constants ----
    ident_f = const.tile([128, 128], F32)
    make_identity(nc, ident_f)
    ident = const.tile([128, 128], F32R)
    nc.vector.tensor_copy(out=ident, in_=ident_f)
    bqs_sb = const.tile([128, HCH], F32)   # 0.125*bq, [d%128, d//128]
    bk_sb = const.tile([128, HCH], F32)
    nc.sync.dma_start(out=bqs_sb, in_=d["bqs"].rearrange("(c p) -> p c", p=128))
    nc.sync.dma_start(out=bk_sb, in_=d["bk"].rearrange("(c p) -> p c", p=128))

    def bcast(src_ap):
        t = const.tile([128, HID], F32)
        nc.sync.dma_start(
            out=t,
            in_=bass.AP(tensor=src_ap.tensor, offset=src_ap.offset,
                        ap=[[0, 128]] + src_ap.ap),
        )
        return t

    bv_b = bcast(d["bv"])
    gamma_b = bcast(d["gamma"])
    beta_b = bcast(d["beta"])
    eps_t = const.tile([128, 1], F32)
    nc.vector.memset(eps_t, EPS)
    maskt_sb = const.tile([128, NB, 2, 128], F16)
    nc.sync.dma_start(out=maskt_sb, in_=d["maskt"].rearrange("t s j i -> j t s i"))
    ones_f = const.tile([128, 2], F32)
    nc.vector.memset(ones_f[:, 0:1], 1.0)
    nc.vector.memset(ones_f[:, 1:2], 0.0)
    ones_r = const.tile([128, 2], F16)
    nc.vector.tensor_copy(out=ones_r, in_=ones_f)

    # ---- stage A: load X (fp32r), build XT via PE transpose ----
    x_all = big.tile([128, NKC, HID], F32R)
    with nc.named_scope("load_x"):
        for sc in range(NKC):
            nc.sync.dma_start(out=x_all[:, sc, :],
                              in_=d["xh"][sc * 128:(sc + 1) * 128, :])
    xt_all = big.tile([128, HCH, KLOC], F32R)
    with nc.named_scope("transpose_x"):
        with tc.tile_pool(name="psT", bufs=6, space="PSUM") as psT:
            for hc in range(HCH):
                for sc in range(NKC):
                    tp = psT.tile([128, 128], F32R, tag="tp", name="tp")
                    nc.tensor.transpose(tp, x_all[:, sc, hc * 128:(hc + 1) * 128],
                                        ident)
                    nc.vector.tensor_copy(
                        out=xt_all[:, hc, sc * 128:(sc + 1) * 128], in_=tp)

    # ---- stage B: projections (fp32r matmuls, fp16 attention operands) ----
    qt_all = big.tile([128, HCH, SLOC], F16)   # [d, dc, own s]
    kt_all = big.tile([128, HCH, KLOC], F16)   # [d, dc, local s]
    v_all = big.tile([128, NKC, NH, HD + 2], F16)  # [s, sc, head, 64+ones+pad]

    with nc.named_scope("proj_q"):
        with tc.tile_pool(name="psQ", bufs=8, space="PSUM") as psQ:
            pq = [psQ.tile([128, SLOC], F32, tag="pq", name="pq")
                  for _ in range(HCH)]
            for hc in range(HCH):
                wq_s = wstream.tile([128, HID], F32R, tag="wq_s", name="wq_s")
                nc.sync.dma_start(out=wq_s, in_=d["wq"][hc * 128:(hc + 1) * 128, :])
                for dc in range(HCH):
                    nc.tensor.matmul(pq[dc], lhsT=wq_s[:, dc * 128:(dc + 1) * 128],
                                     rhs=xt_all[:, hc, HALO:HALO + SLOC],
                                     start=(hc == 0), stop=(hc == HCH - 1))
            for dc in range(HCH):
                nc.vector.tensor_scalar(out=qt_all[:, dc, :], in0=pq[dc],
                                        scalar1=SCALE, scalar2=bqs_sb[:, dc:dc + 1],
                                        op0=mybir.AluOpType.mult,
                                        op1=mybir.AluOpType.add)

    with nc.named_scope("proj_k"):
        with tc.tile_pool(name="psK", bufs=4, space="PSUM") as psK:
            for g in range(2):
                pk = [psK.tile([128, KLOC], F32, tag="pk", name="pk")
                      for _ in range(4)]
                for hc in range(HCH):
                    wk_s = wstream.tile([128, 512], F32R, tag="wk_s", name="wk_s")
                    nc.sync.dma_start(
                        out=wk_s,
                        in_=d["wk"][hc * 128:(hc + 1) * 128, g * 512:(g + 1) * 512])
                    for dci in range(4):
                        nc.tensor.matmul(pk[dci][:, 0:512],
                                         lhsT=wk_s[:, dci * 128:(dci + 1) * 128],
                                         rhs=xt_all[:, hc, 0:512],
                                         start=(hc == 0), stop=(hc == HCH - 1))
                        nc.tensor.matmul(pk[dci][:, 512:KLOC],
                                         lhsT=wk_s[:, dci * 128:(dci + 1) * 128],
                                         rhs=xt_all[:, hc, 512:KLOC],
                                         start=(hc == 0), stop=(hc == HCH - 1))
                for dci in range(4):
                    nc.vector.tensor_scalar_add(out=kt_all[:, 4 * g + dci, :],
                                                in0=pk[dci],
                                                scalar1=bk_sb[:, 4 * g + dci:4 * g + dci + 1])

    with nc.named_scope("proj_v"):
        with tc.tile_pool(name="psV", bufs=4, space="PSUM") as psV:
            for nh_ in range(2):
                wv_t = wres.tile([128, HCH, 512], F32R, tag="wv_t", name="wv_t")
                nc.sync.dma_start(
                    out=wv_t,
                    in_=d["wv"].rearrange("(hc p) d -> p hc d", p=128)
                    [:, :, nh_ * 512:(nh_ + 1) * 512])
                for sc in range(NKC):
                    pv = psV.tile([128, 512], F32, tag="pv", name="pv")
                    for hc in range(HCH):
                        nc.tensor.matmul(pv,
                                         lhsT=xt_all[:, hc, sc * 128:(sc + 1) * 128],
                                         rhs=wv_t[:, hc, :],
                                         start=(hc == 0), stop=(hc == HCH - 1))
                    nc.vector.tensor_add(
                        out=v_all[:, sc, nh_ * 8:(nh_ + 1) * 8, 0:HD],
                        in0=pv[:].rearrange("p (h e) -> p h e", e=HD),
                        in1=bv_b[:, nh_ * 512:(nh_ + 1) * 512]
                        .rearrange("p (h e) -> p h e", e=HD))
            for sc in range(NKC):
                nc.vector.tensor_copy(
                    out=v_all[:, sc, :, HD:HD + 2],
                    in_=ones_r.unsqueeze(1).to_broadcast([128, NH, 2]))

    # ---- stage C: attention + stage D: residual/LN ----
    psS = ctx.enter_context(tc.tile_pool(name="psS", bufs=3, space="PSUM"))
    psC = ctx.enter_context(tc.tile_pool(name="psC", bufs=2, space="PSUM"))
    ctx_sb = {}
    for p in range(2):                      # pairs of query blocks
        for t in (2 * p, 2 * p + 1):
            ctx_sb[t] = ctx_p.tile([128, NH, HD], F32, tag="ctx_sb", name="ctx_sb")
        cps = {}
        for h in range(NH):
            dc, ph = h // 2, (h % 2) * 64
            pscore = psS.tile([128, 4, 256], F32, tag="pscore", name="pscore")
            with nc.named_scope("scores"):
                for m in range(4):
                    nc.tensor.matmul(
                        pscore[:, m, :],
                        lhsT=kt_all[ph:ph + 64, dc, (2 * p + m) * 128:(2 * p + m + 1) * 128],
                        rhs=qt_all[ph:ph + 64, dc, p * 256:(p + 1) * 256],
                        start=True, stop=True)
            for q in range(2):
                t = 2 * p + q
                em = expm_p.tile([128, 3, 128], F16, tag="expm", name="em")
                with nc.named_scope("softmax"):
                    nc.scalar.activation(
                        out=em, in_=pscore[:, q:q + 3, q * 128:(q + 1) * 128],
                        func=AFT.Exp)
                    nc.vector.tensor_mul(
                        out=em[:, 0::2, :], in0=em[:, 0::2, :],
                        in1=maskt_sb[:, t, :, :])
                g, hi = h // 4, h % 4
                if hi == 0:
                    cps[(q, g)] = psC.tile([128, 4, HD + 2], F32, tag="cps",
                                           name="cps")
                with nc.named_scope("pv"):
                    for c in range(3):
                        nc.tensor.matmul(
                            cps[(q, g)][:, hi, :],
                            lhsT=em[:, c, :],
                            rhs=v_all[:, t + c, h, :],
                            start=(c == 0), stop=(c == 2))
                if hi == 3:
                    with nc.named_scope("ctx_scale"):
                        zv = small.tile([128, 4], F32, tag="zv", name="zv")
                        nc.vector.tensor_copy(out=zv, in_=cps[(q, g)][:, :, HD])
                        nc.vector.reciprocal(out=zv, in_=zv)
                        nc.vector.tensor_mul(
                            out=ctx_sb[t][:, 4 * g:4 * g + 4, :],
                            in0=cps[(q, g)][:, :, 0:HD],
                            in1=zv.unsqueeze(2).to_broadcast([128, 4, HD]))

        for q in range(2):
            t = 2 * p + q
            with nc.named_scope("layernorm"):
                xs = temps.tile([128, HID], F32, tag="xs", name="xs")
                nc.vector.tensor_add(
                    out=xs, in0=x_all[:, t + 1, :].bitcast(F32),
                    in1=ctx_sb[t][:].rearrange("p h e -> p (h e)"))
                stats = small.tile([128, 2, 6], F32, tag="stats", name="stats")
                for sg in range(2):
                    nc.vector.bn_stats(out=stats[:, sg, :],
                                       in_=xs[:, sg * 512:(sg + 1) * 512])
                mv = small.tile([128, 2], F32, tag="mv", name="mv")
                nc.vector.bn_aggr(out=mv, in_=stats)
                rstd = small.tile([128, 1], F32, tag="rstd", name="rstd")
                nc.scalar.activation(out=rstd, in_=mv[:, 1:2], func=AFT.Sqrt,
                                     bias=eps_t)
                nc.vector.reciprocal(out=rstd, in_=rstd)
                xn = temps.tile([128, HID], F32, tag="xn", name="xn")
                nc.vector.tensor_scalar(out=xn, in0=xs, scalar1=mv[:, 0:1],
                                        scalar2=rstd,
                                        op0=mybir.AluOpType.subtract,
                                        op1=mybir.AluOpType.mult)
                nc.gpsimd.tensor_mul(out=xn, in0=xn, in1=gamma_b)
                nc.gpsimd.tensor_add(out=xn, in0=xn, in1=beta_b)
                nc.sync.dma_start(out=d["out"][t * 128:(t + 1) * 128, :], in_=xn)


def build_module():
    nc = bacc.Bacc("TRN2", target_bir_lowering=False, debug=False,
                   num_devices=NCORES)
    d = {
        "xh": nc.dram_tensor("xh", [KLOC, HID], F32R, kind="ExternalInput").ap(),
        "wq": nc.dram_tensor("wq", [HID, HID], F32R, kind="ExternalInput").ap(),
        "wk": nc.dram_tensor("wk", [HID, HID], F32R, kind="ExternalInput").ap(),
        "wv": nc.dram_tensor("wv", [HID, HID], F32R, kind="ExternalInput").ap(),
        "bqs": nc.dram_tensor("bqs", [HID], F32, kind="ExternalInput").ap(),
        "bk": nc.dram_tensor("bk", [HID], F32, kind="ExternalInput").ap(),
        "bv": nc.dram_tensor("bv", [HID], F32, kind="ExternalInput").ap(),
        "gamma": nc.dram_tensor("gamma", [HID], F32, kind="ExternalInput").ap(),
        "beta": nc.dram_tensor("beta", [HID], F32, kind="ExternalInput").ap(),
        "maskt": nc.dram_tensor("maskt", [NB, 2, 128, 128], mybir.dt.float16,
                                kind="ExternalInput").ap(),
        "out": nc.dram_tensor("out", [SLOC, HID], F32, kind="ExternalOutput").ap(),
    }
    from contextlib import ExitStack
    with tile.TileContext(nc) as tc:
        with ExitStack() as ctx:
            _emit(nc, tc, ctx, d)
    nc.compile()
    return nc


def _make_masks():
    """maskt[core][t, side, jc, i]: 1.0 keep / 0.0 drop, scoresT orientation."""
    jc = np.arange(128)[:, None]
    i = np.arange(128)[None, :]
    band = [jc >= i, jc < i]              # side 0: chunk m=0; side 1: chunk m=2
    masks = np.zeros((NCORES, NB, 2, 128, 128), np.float32)
    for c in range(NCORES):
        for t in range(NB):
            k0 = c * SLOC + t * 128 - HALO     # global j of local chunk col 0
            for side, m in ((0, 0), (1, 2)):
                jg = k0 + m * 128 + jc
                valid = (jg >= 0) & (jg < S)
                masks[c, t, side] = (band[side] & valid).astype(np.float32)
    return masks


_STATE = {}


def kernel(**inputs):
    hs = np.asarray(inputs["hidden_states"], np.float32).reshape(S, HID)
    wq = np.ascontiguousarray(np.asarray(inputs["Wq"], np.float32))
    wk = np.ascontiguousarray(np.asarray(inputs["Wk"], np.float32))
    wv = np.ascontiguousarray(np.asarray(inputs["Wv"], np.float32))
    bq = np.asarray(inputs["bq"], np.float32)
    bk = np.asarray(inputs["bk"], np.float32)
    bv = np.asarray(inputs["bv"], np.float32)
    gamma = np.asarray(inputs["gamma"], np.float32)
    beta = np.asarray(inputs["beta"], np.float32)

    if "nc" not in _STATE:
        _STATE["nc"] = build_module()
        _STATE["masks"] = _make_masks()
    nc = _STATE["nc"]
    masks = _STATE["masks"]

    xpad = np.zeros((S + 2 * HALO, HID), np.float32)
    xpad[HALO:HALO + S] = hs
    common = {"wq": wq, "wk": wk, "wv": wv, "bqs": (SCALE * bq).astype(np.float32),
              "bk": bk, "bv": bv, "gamma": gamma, "beta": beta}
    in_maps = [
        {**common, "xh": np.ascontiguousarray(xpad[c * SLOC:c * SLOC + KLOC]),
         "maskt": np.ascontiguousarray(masks[c].astype(np.float16))}
        for c in range(NCORES)
    ]
    res = bass_utils.run_bass_kernel_spmd(nc, in_maps,
                                          core_ids=list(range(NCORES)),
                                          **_STATE.get("run_kwargs", {}))
    _STATE["last_result"] = res
    out = np.concatenate([res.results[c]["out"] for c in range(NCORES)], axis=0)
    return out.reshape(1, S, HID)


# revision 12
# speedup vs baseline: 1.4272x; 1.0479x over previous
"""Sliding-window attention + residual + LayerNorm on 8 Trainium2 NeuronCores.

Problem (hardcoded): B=1, S=4096, HID=1024, NH=16, HD=64, WIN=256.
    q,k,v = X@W* + b*  (per-head HD=64)
    scores = q k^T / 8, sliding-window mask (j in [i-128, i+128)), softmax
    out = LayerNorm(X + probs@v) * gamma + beta

Sharding: sequence-parallel. Core c owns query rows [c*512, c*512+512) and
receives X rows [c*512-128, c*512+640) (zero-padded at the sequence edges) so
all K/V it attends to are computed locally (halo recompute, no collectives).

Per-core kernel layout (all SBUF tiles are [128 partitions, ...]):
  - X is PE-transposed once into XT [h, s] (fp32r) and reused by all three
    projections.
  - QT/KT are produced transposed ([d, s], head-major: head h = d-chunk h//2,
    partition half h%2) directly from the projection matmul; V stays natural
    [s, d] with a ones-column appended per head (V_aug [s, 65]) so the PV
    matmul also produces the softmax denominator Z for free.
  - scores are computed TRANSPOSED (scoresT[j, i] = kT.T @ qT) so no
    transpose of probabilities is ever needed; softmax skips max-subtraction
    (|scores| <= ~8, exp cannot overflow; masked entries are multiplied by 0
    after exp which matches the reference's exp(-10000-max) underflow).
  - matmuls run in float32r (TF32-like, ~1.5e-4 rel err, 4x faster than fp32).
"""

import numpy as np

import concourse.bass as bass
import concourse.tile as tile
from concourse import bacc, mybir
from concourse import bass_utils
from concourse.masks import make_identity

F32 = mybir.dt.float32
F32R = mybir.dt.float32r
F16 = mybir.dt.float16
AFT = mybir.ActivationFunctionType

S, HID, NH, HD = 4096, 1024, 16, 64
WIN = 256
EPS = 1e-12
NCORES = 8
SLOC = S // NCORES          # 512 own rows per core
HALO = WIN // 2             # 128
KLOC = SLOC + 2 * HALO      # 768 local K/V rows
NB = SLOC // 128            # 4 query blocks per core
NKC = KLOC // 128           # 6 local K chunks
HCH = HID // 128            # 8 hidden chunks
SCALE = 1.0 / np.sqrt(HD)


def _emit(nc, tc, ctx, d):
    """Emit the per-core kernel into TileContext tc. d: dict of DRAM APs."""
    const = ctx.enter_context(tc.tile_pool(name="const", bufs=1))
    big = ctx.enter_context(tc.tile_pool(name="big", bufs=1))
    wstream = ctx.enter_context(tc.tile_pool(name="wstream", bufs=3))
    wres = ctx.enter_context(tc.tile_pool(name="wres", bufs=2))
    expm_p = ctx.enter_context(tc.tile_pool(name="expm", bufs=8))
    temps = ctx.enter_context(tc.tile_pool(name="temps", bufs=4))
    small = ctx.enter_context(tc.tile_pool(name="small", bufs=8))
    ctx_p = ctx.enter_context(tc.tile_pool(name="ctxp", bufs=2))

    # ---- constants ----
    ident_f = const.tile([128, 128], F32)
    make_identity(nc, ident_f)
    ident = const.tile([128, 128], F32R)
    nc.vector.tensor_copy(out=ident, in_=ident_f)
    bqs_sb = const.tile([128, HCH], F32)   # 0.125*bq, [d%128, d//128]
    bk_sb = const.tile([128, HCH], F32)
    nc.sync.dma_start(out=bqs_sb, in_=d["bqs"].rearrange("(c p) -> p c", p=128))
    nc.sync.dma_start(out=bk_sb, in_=d["bk"].rearrange("(c p) -> p c", p=128))

    def bcast(src_ap):
        t = const.tile([128, HID], F32)
        nc.sync.dma_start(
            out=t,
            in_=bass.AP(tensor=src_ap.tensor, offset=src_ap.offset,
                        ap=[[0, 128]] + src_ap.ap),
        )
        return t

    bv_b = bcast(d["bv"])
    gamma_b = bcast(d["gamma"])
    beta_b = bcast(d["beta"])
    eps_t = const.tile([128, 1], F32)
    nc.vector.memset(eps_t, EPS)
    maskt_sb = const.tile([128, NB, 2, 128], F16)
    nc.sync.dma_start(out=maskt_sb, in_=d["maskt"].rearrange("t s j i -> j t s i"))
    ones_f = const.tile([128, 2], F32)
    nc.vector.memset(ones_f[:, 0:1], 1.0)
    nc.vector.memset(ones_f[:, 1:2], 0.0)
    ones_r = const.tile([128, 2], F16)
    nc.vector.tensor_copy(out=ones_r, in_=ones_f)

    # ---- stage A: load X (fp32r), build XT via PE transpose ----
    x_all = big.tile([128, NKC, HID], F32R)
    with nc.named_scope("load_x"):
        for sc in range(NKC):
            nc.sync.dma_start(out=x_all[:, sc, :],
                              in_=d["xh"][sc * 128:(sc + 1) * 128, :])
    xt_all = big.tile([128, HCH, KLOC], F32R)
    with nc.named_scope("transpose_x"):
        with tc.tile_pool(name="psT", bufs=6, space="PSUM") as psT:
            for hc in range(HCH):
                for sc in range(NKC):
                    tp = psT.tile([128, 128], F32R, tag="tp", name="tp")
                    nc.tensor.transpose(tp, x_all[:, sc, hc * 128:(hc + 1) * 128],
                                        ident)
                    nc.vector.tensor_copy(
                        out=xt_all[:, hc, sc * 128:(sc + 1) * 128], in_=tp)

    # ---- stage B: projections (fp32r matmuls, fp16 attention operands) ----
    qt_all = big.tile([128, HCH, SLOC], F16)   # [d, dc, own s]
    kt_all = big.tile([128, HCH, KLOC], F16)   # [d, dc, local s]
    v_all = big.tile([128, NKC, NH, HD + 2], F16)  # [s, sc, head, 64+ones+pad]

    with nc.named_scope("proj_q"):
        with tc.tile_pool(name="psQ", bufs=8, space="PSUM") as psQ:
            pq = [psQ.tile([128, SLOC], F32, tag="pq", name="pq")
                  for _ in range(HCH)]
            for hc in range(HCH):
                wq_s = wstream.tile([128, HID], F32R, tag="wq_s", name="wq_s")
                nc.sync.dma_start(out=wq_s, in_=d["wq"][hc * 128:(hc + 1) * 128, :])
                for dc in range(HCH):
                    nc.tensor.matmul(pq[dc], lhsT=wq_s[:, dc * 128:(dc + 1) * 128],
                                     rhs=xt_all[:, hc, HALO:HALO + SLOC],
                                     start=(hc == 0), stop=(hc == HCH - 1))
            for dc in range(HCH):
                nc.vector.tensor_scalar(out=qt_all[:, dc, :], in0=pq[dc],
                                        scalar1=SCALE, scalar2=bqs_sb[:, dc:dc + 1],
                                        op0=mybir.AluOpType.mult,
                                        op1=mybir.AluOpType.add)

    with nc.named_scope("proj_k"):
        with tc.tile_pool(name="psK", bufs=4, space="PSUM") as psK:
            for g in range(2):
                pk = [psK.tile([128, KLOC], F32, tag="pk", name="pk")
                      for _ in range(4)]
                for hc in range(HCH):
                    wk_s = wstream.tile([128, 512], F32R, tag="wk_s", name="wk_s")
                    nc.sync.dma_start(
                        out=wk_s,
                        in_=d["wk"][hc * 128:(hc + 1) * 128, g * 512:(g + 1) * 512])
                    for dci in range(4):
                        nc.tensor.matmul(pk[dci][:, 0:512],
                                         lhsT=wk_s[:, dci * 128:(dci + 1) * 128],
                                         rhs=xt_all[:, hc, 0:512],
                                         start=(hc == 0), stop=(hc == HCH - 1))
                        nc.tensor.matmul(pk[dci][:, 512:KLOC],
                                         lhsT=wk_s[:, dci * 128:(dci + 1) * 128],
                                         rhs=xt_all[:, hc, 512:KLOC],
                                         start=(hc == 0), stop=(hc == HCH - 1))
                for dci in range(4):
                    nc.vector.tensor_scalar_add(out=kt_all[:, 4 * g + dci, :],
                                                in0=pk[dci],
                                                scalar1=bk_sb[:, 4 * g + dci:4 * g + dci + 1])

    with nc.named_scope("proj_v"):
        with tc.tile_pool(name="psV", bufs=4, space="PSUM") as psV:
            for nh_ in range(2):
                wv_t = wres.tile([128, HCH, 512], F32R, tag="wv_t", name="wv_t")
                nc.sync.dma_start(
                    out=wv_t,
                    in_=d["wv"].rearrange("(hc p) d -> p hc d", p=128)
                    [:, :, nh_ * 512:(nh_ + 1) * 512])
                for sc in range(NKC):
                    pv = psV.tile([128, 512], F32, tag="pv", name="pv")
                    for hc in range(HCH):
                        nc.tensor.matmul(pv,
                                         lhsT=xt_all[:, hc, sc * 128:(sc + 1) * 128],
                                         rhs=wv_t[:, hc, :],
                                         start=(hc == 0), stop=(hc == HCH - 1))
                    nc.vector.tensor_add(
                        out=v_all[:, sc, nh_ * 8:(nh_ + 1) * 8, 0:HD],
                        in0=pv[:].rearrange("p (h e) -> p h e", e=HD),
                        in1=bv_b[:, nh_ * 512:(nh_ + 1) * 512]
                        .rearrange("p (h e) -> p h e", e=HD))
            for sc in range(NKC):
                nc.vector.tensor_copy(
                    out=v_all[:, sc, :, HD:HD + 2],
                    in_=ones_r.unsqueeze(1).to_broadcast([128, NH, 2]))

    # ---- stage C: attention + stage D: residual/LN ----
    psS = ctx.enter_context(tc.tile_pool(name="psS", bufs=2, space="PSUM"))
    psC = ctx.enter_context(tc.tile_pool(name="psC", bufs=4, space="PSUM"))
    ctx_sb = {}
    for p in range(2):                      # pairs of query blocks
        for t in (2 * p, 2 * p + 1):
            ctx_sb[t] = ctx_p.tile([128, NH, HD], F32, tag="ctx_sb", name="ctx_sb")
        cps = {}
        for hp in range(NH // 2):           # head pairs (2*hp, 2*hp+1)
            dc = hp
            psc = {}
            with nc.named_scope("scores"):
                for m in range(4):
                    for ho in range(2):     # interleave partition-halves -> PE row tiling
                        h = 2 * hp + ho
                        if m == 0:
                            psc[h] = psS.tile([128, 4, 256], F32, tag="pscore",
                                              name="pscore")
                        ph = ho * 64
                        nc.tensor.matmul(
                            psc[h][:, m, :],
                            lhsT=kt_all[ph:ph + 64, dc, (2 * p + m) * 128:(2 * p + m + 1) * 128],
                            rhs=qt_all[ph:ph + 64, dc, p * 256:(p + 1) * 256],
                            start=True, stop=True)
            for ho in range(2):
                h = 2 * hp + ho
                em = expm_p.tile([128, 4, 256], F16, tag="expm", name="em")
                with nc.named_scope("softmax"):
                    nc.scalar.activation(out=em, in_=psc[h], func=AFT.Exp)
                    for q in range(2):
                        t = 2 * p + q
                        nc.vector.tensor_mul(
                            out=em[:, q::2, q * 128:(q + 1) * 128],
                            in0=em[:, q::2, q * 128:(q + 1) * 128],
                            in1=maskt_sb[:, t, :, :])
                g, hi = h // 4, h % 4
                for q in range(2):
                    t = 2 * p + q
                    if hi == 0 and q == 0:
                        cps[(0, g)] = psC.tile([128, 4, HD + 2], F32, tag="cps",
                                               name="cps")
                        cps[(1, g)] = psC.tile([128, 4, HD + 2], F32, tag="cps",
                                               name="cps")
                    with nc.named_scope("pv"):
                        for c in range(3):
                            nc.tensor.matmul(
                                cps[(q, g)][:, hi, :],
                                lhsT=em[:, q + c, q * 128:(q + 1) * 128],
                                rhs=v_all[:, t + c, h, :],
                                start=(c == 0), stop=(c == 2))
                if hi == 3:
                    for q in range(2):
                        t = 2 * p + q
                        with nc.named_scope("ctx_scale"):
                            zv = small.tile([128, 4], F32, tag="zv", name="zv")
                            nc.vector.tensor_copy(out=zv, in_=cps[(q, g)][:, :, HD])
                            nc.vector.reciprocal(out=zv, in_=zv)
                            nc.vector.tensor_mul(
                                out=ctx_sb[t][:, 4 * g:4 * g + 4, :],
                                in0=cps[(q, g)][:, :, 0:HD],
                                in1=zv.unsqueeze(2).to_broadcast([128, 4, HD]))

        for q in range(2):
            t = 2 * p + q
            with nc.named_scope("layernorm"):
                xs = temps.tile([128, HID], F32, tag="xs", name="xs")
                nc.vector.tensor_add(
                    out=xs, in0=x_all[:, t + 1, :].bitcast(F32),
                    in1=ctx_sb[t][:].rearrange("p h e -> p (h e)"))
                stats = small.tile([128, 2, 6], F32, tag="stats", name="stats")
                for sg in range(2):
                    nc.vector.bn_stats(out=stats[:, sg, :],
                                       in_=xs[:, sg * 512:(sg + 1) * 512])
                mv = small.tile([128, 2], F32, tag="mv", name="mv")
                nc.vector.bn_aggr(out=mv, in_=stats)
                rstd = small.tile([128, 1], F32, tag="rstd", name="rstd")
                nc.scalar.activation(out=rstd, in_=mv[:, 1:2], func=AFT.Sqrt,
                                     bias=eps_t)
                nc.vector.reciprocal(out=rstd, in_=rstd)
                xn = temps.tile([128, HID], F32, tag="xn", name="xn")
                nc.vector.tensor_scalar(out=xn, in0=xs, scalar1=mv[:, 0:1],
                                        scalar2=rstd,
                                        op0=mybir.AluOpType.subtract,
                                        op1=mybir.AluOpType.mult)
                nc.vector.tensor_mul(out=xn, in0=xn, in1=gamma_b)
                nc.vector.tensor_add(out=xn, in0=xn, in1=beta_b)
                nc.sync.dma_start(out=d["out"][t * 128:(t + 1) * 128, :], in_=xn)


def build_module():
    nc = bacc.Bacc("TRN2", target_bir_lowering=False, debug=False,
                   num_devices=NCORES)
    d = {
        "xh": nc.dram_tensor("xh", [KLOC, HID], F32R, kind="ExternalInput").ap(),
        "wq": nc.dram_tensor("wq", [HID, HID], F32R, kind="ExternalInput").ap(),
        "wk": nc.dram_tensor("wk", [HID, HID], F32R, kind="ExternalInput").ap(),
        "wv": nc.dram_tensor("wv", [HID, HID], F32R, kind="ExternalInput").ap(),
        "bqs": nc.dram_tensor("bqs", [HID], F32, kind="ExternalInput").ap(),
        "bk": nc.dram_tensor("bk", [HID], F32, kind="ExternalInput").ap(),
        "bv": nc.dram_tensor("bv", [HID], F32, kind="ExternalInput").ap(),
        "gamma": nc.dram_tensor("gamma", [HID], F32, kind="ExternalInput").ap(),
        "beta": nc.dram_tensor("beta", [HID], F32, kind="ExternalInput").ap(),
        "maskt": nc.dram_tensor("maskt", [NB, 2, 128, 128], mybir.dt.float16,
                                kind="ExternalInput").ap(),
        "out": nc.dram_tensor("out", [SLOC, HID], F32, kind="ExternalOutput").ap(),
    }
    from contextlib import ExitStack
    with tile.TileContext(nc) as tc:
        with ExitStack() as ctx:
            _emit(nc, tc, ctx, d)
    nc.compile()
    return nc


def _make_masks():
    """maskt[core][t, side, jc, i]: 1.0 keep / 0.0 drop, scoresT orientation."""
    jc = np.arange(128)[:, None]
    i = np.arange(128)[None, :]
    band = [jc >= i, jc < i]              # side 0: chunk m=0; side 1: chunk m=2
    masks = np.zeros((NCORES, NB, 2, 128, 128), np.float32)
    for c in range(NCORES):
        for t in range(NB):
            k0 = c * SLOC + t * 128 - HALO     # global j of local chunk col 0
            for side, m in ((0, 0), (1, 2)):
                jg = k0 + m * 128 + jc
                valid = (jg >= 0) & (jg < S)
                masks[c, t, side] = (band[side] & valid).astype(np.float32)
    return masks


_STATE = {}


def kernel(**inputs):
    hs = np.asarray(inputs["hidden_states"], np.float32).reshape(S, HID)
    wq = np.ascontiguousarray(np.asarray(inputs["Wq"], np.float32))
    wk = np.ascontiguousarray(np.asarray(inputs["Wk"], np.float32))
    wv = np.ascontiguousarray(np.asarray(inputs["Wv"], np.float32))
    bq = np.asarray(inputs["bq"], np.float32)
    bk = np.asarray(inputs["bk"], np.float32)
    bv = np.asarray(inputs["bv"], np.float32)
    gamma = np.asarray(inputs["gamma"], np.float32)
    beta = np.asarray(inputs["beta"], np.float32)

    if "nc" not in _STATE:
        _STATE["nc"] = build_module()
        _STATE["masks"] = _make_masks()
    nc = _STATE["nc"]
    masks = _STATE["masks"]

    xpad = np.zeros((S + 2 * HALO, HID), np.float32)
    xpad[HALO:HALO + S] = hs
    common = {"wq": wq, "wk": wk, "wv": wv, "bqs": (SCALE * bq).astype(np.float32),
              "bk": bk, "bv": bv, "gamma": gamma, "beta": beta}
    in_maps = [
        {**common, "xh": np.ascontiguousarray(xpad[c * SLOC:c * SLOC + KLOC]),
         "maskt": np.ascontiguousarray(masks[c].astype(np.float16))}
        for c in range(NCORES)
    ]
    res = bass_utils.run_bass_kernel_spmd(nc, in_maps,
                                          core_ids=list(range(NCORES)),
                                          **_STATE.get("run_kwargs", {}))
    _STATE["last_result"] = res
    out = np.concatenate([res.results[c]["out"] for c in range(NCORES)], axis=0)
    return out.reshape(1, S, HID)


# revision 13
# speedup vs baseline: 1.5043x; 1.0541x over previous
"""Sliding-window attention + residual + LayerNorm on 8 Trainium2 NeuronCores.

Problem (hardcoded): B=1, S=4096, HID=1024, NH=16, HD=64, WIN=256.
    q,k,v = X@W* + b*  (per-head HD=64)
    scores = q k^T / 8, sliding-window mask (j in [i-128, i+128)), softmax
    out = LayerNorm(X + probs@v) * gamma + beta

Sharding: sequence-parallel. Core c owns query rows [c*512, c*512+512) and
receives X rows [c*512-128, c*512+640) (zero-padded at the sequence edges) so
all K/V it attends to are computed locally (halo recompute, no collectives).

Per-core kernel layout (all SBUF tiles are [128 partitions, ...]):
  - X is PE-transposed once into XT [h, s] (fp32r) and reused by all three
    projections.
  - QT/KT are produced transposed ([d, s], head-major: head h = d-chunk h//2,
    partition half h%2) directly from the projection matmul; V stays natural
    [s, d] with a ones-column appended per head (V_aug [s, 65]) so the PV
    matmul also produces the softmax denominator Z for free.
  - scores are computed TRANSPOSED (scoresT[j, i] = kT.T @ qT) so no
    transpose of probabilities is ever needed; softmax skips max-subtraction
    (|scores| <= ~8, exp cannot overflow; masked entries are multiplied by 0
    after exp which matches the reference's exp(-10000-max) underflow).
  - matmuls run in float32r (TF32-like, ~1.5e-4 rel err, 4x faster than fp32).
"""

import numpy as np

import concourse.bass as bass
import concourse.tile as tile
from concourse import bacc, mybir
from concourse import bass_utils
from concourse.masks import make_identity

F32 = mybir.dt.float32
F32R = mybir.dt.float32r
F16 = mybir.dt.float16
AFT = mybir.ActivationFunctionType

S, HID, NH, HD = 4096, 1024, 16, 64
WIN = 256
EPS = 1e-12
NCORES = 8
SLOC = S // NCORES          # 512 own rows per core
HALO = WIN // 2             # 128
KLOC = SLOC + 2 * HALO      # 768 local K/V rows
NB = SLOC // 128            # 4 query blocks per core
NKC = KLOC // 128           # 6 local K chunks
HCH = HID // 128            # 8 hidden chunks
SCALE = 1.0 / np.sqrt(HD)


def _emit(nc, tc, ctx, d):
    """Emit the per-core kernel into TileContext tc. d: dict of DRAM APs."""
    const = ctx.enter_context(tc.tile_pool(name="const", bufs=1))
    big = ctx.enter_context(tc.tile_pool(name="big", bufs=1))
    wstream = ctx.enter_context(tc.tile_pool(name="wstream", bufs=3))
    wres = ctx.enter_context(tc.tile_pool(name="wres", bufs=2))
    expm_p = ctx.enter_context(tc.tile_pool(name="expm", bufs=8))
    temps = ctx.enter_context(tc.tile_pool(name="temps", bufs=4))
    small = ctx.enter_context(tc.tile_pool(name="small", bufs=8))
    ctx_p = ctx.enter_context(tc.tile_pool(name="ctxp", bufs=2))

    # ---- constants ----
    ident_f = const.tile([128, 128], F32)
    make_identity(nc, ident_f)
    ident = const.tile([128, 128], F32R)
    nc.vector.tensor_copy(out=ident, in_=ident_f)
    bqs_sb = const.tile([128, HCH], F32)   # 0.125*bq, [d%128, d//128]
    bk_sb = const.tile([128, HCH], F32)
    nc.sync.dma_start(out=bqs_sb, in_=d["bqs"].rearrange("(c p) -> p c", p=128))
    nc.sync.dma_start(out=bk_sb, in_=d["bk"].rearrange("(c p) -> p c", p=128))

    def bcast(src_ap):
        t = const.tile([128, HID], F32)
        nc.sync.dma_start(
            out=t,
            in_=bass.AP(tensor=src_ap.tensor, offset=src_ap.offset,
                        ap=[[0, 128]] + src_ap.ap),
        )
        return t

    bv_b = bcast(d["bv"])
    gamma_b = bcast(d["gamma"])
    beta_b = bcast(d["beta"])
    eps_t = const.tile([128, 1], F32)
    nc.vector.memset(eps_t, EPS)
    maskt_sb = const.tile([128, NB, 2, 128], F16)
    nc.sync.dma_start(out=maskt_sb, in_=d["maskt"].rearrange("t s j i -> j t s i"))
    ones_f = const.tile([128, 2], F32)
    nc.vector.memset(ones_f[:, 0:1], 1.0)
    nc.vector.memset(ones_f[:, 1:2], 0.0)
    ones_r = const.tile([128, 2], F16)
    nc.vector.tensor_copy(out=ones_r, in_=ones_f)

    # ---- stage A: load X (fp32r), build XT via PE transpose ----
    x_all = big.tile([128, NKC, HID], F32R)
    with nc.named_scope("load_x"):
        for sc in range(NKC):
            nc.sync.dma_start(out=x_all[:, sc, :],
                              in_=d["xh"][sc * 128:(sc + 1) * 128, :])
    xt_all = big.tile([128, HCH, KLOC], F32R)
    with nc.named_scope("transpose_x"):
        with tc.tile_pool(name="psT", bufs=6, space="PSUM") as psT:
            for hc in range(HCH):
                for sc in range(NKC):
                    tp = psT.tile([128, 128], F32R, tag="tp", name="tp")
                    nc.tensor.transpose(tp, x_all[:, sc, hc * 128:(hc + 1) * 128],
                                        ident)
                    nc.vector.tensor_copy(
                        out=xt_all[:, hc, sc * 128:(sc + 1) * 128], in_=tp)

    # ---- stage B: projections (fp32r matmuls, fp16 attention operands) ----
    qt_all = big.tile([128, HCH, SLOC], F16)   # [d, dc, own s]
    kt_all = big.tile([128, HCH, KLOC], F16)   # [d, dc, local s]
    v_all = big.tile([128, NKC, NH, HD + 2], F16)  # [s, sc, head, 64+ones+pad]

    with nc.named_scope("proj_q"):
        with tc.tile_pool(name="psQ", bufs=8, space="PSUM") as psQ:
            pq = [psQ.tile([128, SLOC], F32, tag="pq", name="pq")
                  for _ in range(HCH)]
            for hc in range(HCH):
                wq_s = wstream.tile([128, HID], F32R, tag="wq_s", name="wq_s")
                nc.sync.dma_start(out=wq_s, in_=d["wq"][hc * 128:(hc + 1) * 128, :])
                for dc in range(HCH):
                    nc.tensor.matmul(pq[dc], lhsT=wq_s[:, dc * 128:(dc + 1) * 128],
                                     rhs=xt_all[:, hc, HALO:HALO + SLOC],
                                     start=(hc == 0), stop=(hc == HCH - 1))
            for dc in range(HCH):
                nc.vector.tensor_scalar(out=qt_all[:, dc, :], in0=pq[dc],
                                        scalar1=SCALE, scalar2=bqs_sb[:, dc:dc + 1],
                                        op0=mybir.AluOpType.mult,
                                        op1=mybir.AluOpType.add)

    with nc.named_scope("proj_k"):
        with tc.tile_pool(name="psK", bufs=4, space="PSUM") as psK:
            for g in range(2):
                pk = [psK.tile([128, KLOC], F32, tag="pk", name="pk")
                      for _ in range(4)]
                for hc in range(HCH):
                    wk_s = wstream.tile([128, 512], F32R, tag="wk_s", name="wk_s")
                    nc.sync.dma_start(
                        out=wk_s,
                        in_=d["wk"][hc * 128:(hc + 1) * 128, g * 512:(g + 1) * 512])
                    for dci in range(4):
                        nc.tensor.matmul(pk[dci][:, 0:512],
                                         lhsT=wk_s[:, dci * 128:(dci + 1) * 128],
                                         rhs=xt_all[:, hc, 0:512],
                                         start=(hc == 0), stop=(hc == HCH - 1))
                        nc.tensor.matmul(pk[dci][:, 512:KLOC],
                                         lhsT=wk_s[:, dci * 128:(dci + 1) * 128],
                                         rhs=xt_all[:, hc, 512:KLOC],
                                         start=(hc == 0), stop=(hc == HCH - 1))
                for dci in range(4):
                    nc.vector.tensor_scalar_add(out=kt_all[:, 4 * g + dci, :],
                                                in0=pk[dci],
                                                scalar1=bk_sb[:, 4 * g + dci:4 * g + dci + 1])

    with nc.named_scope("proj_v"):
        with tc.tile_pool(name="psV", bufs=4, space="PSUM") as psV:
            for nh_ in range(2):
                wv_t = wres.tile([128, HCH, 512], F32R, tag="wv_t", name="wv_t")
                nc.sync.dma_start(
                    out=wv_t,
                    in_=d["wv"].rearrange("(hc p) d -> p hc d", p=128)
                    [:, :, nh_ * 512:(nh_ + 1) * 512])
                for sc in range(NKC):
                    pv = psV.tile([128, 512], F32, tag="pv", name="pv")
                    for hc in range(HCH):
                        nc.tensor.matmul(pv,
                                         lhsT=xt_all[:, hc, sc * 128:(sc + 1) * 128],
                                         rhs=wv_t[:, hc, :],
                                         start=(hc == 0), stop=(hc == HCH - 1))
                    nc.vector.tensor_add(
                        out=v_all[:, sc, nh_ * 8:(nh_ + 1) * 8, 0:HD],
                        in0=pv[:].rearrange("p (h e) -> p h e", e=HD),
                        in1=bv_b[:, nh_ * 512:(nh_ + 1) * 512]
                        .rearrange("p (h e) -> p h e", e=HD))
            for sc in range(NKC):
                nc.vector.tensor_copy(
                    out=v_all[:, sc, :, HD:HD + 2],
                    in_=ones_r.unsqueeze(1).to_broadcast([128, NH, 2]))

    # ---- stage C: attention + stage D: residual/LN ----
    psS = ctx.enter_context(tc.tile_pool(name="psS", bufs=6, space="PSUM"))
    psC = ctx.enter_context(tc.tile_pool(name="psC", bufs=2, space="PSUM"))
    for t in range(NB):
        ctx_t = ctx_p.tile([128, NH, HD], F32, tag="ctx_sb", name="ctx_sb")
        cps = {}
        for hp in range(NH // 2):           # head pairs (2*hp, 2*hp+1)
            dc = hp
            psc = {}
            with nc.named_scope("scores"):
                for c in range(3):
                    for ho in range(2):     # partition halves -> PE row tiling
                        h = 2 * hp + ho
                        if c == 0:
                            psc[h] = psS.tile([128, 3, 128], F32, tag="pscore",
                                              name="pscore")
                        ph = ho * 64
                        nc.tensor.matmul(
                            psc[h][:, c, :],
                            lhsT=kt_all[ph:ph + 64, dc, (t + c) * 128:(t + c + 1) * 128],
                            rhs=qt_all[ph:ph + 64, dc, t * 128:(t + 1) * 128],
                            start=True, stop=True)
            for ho in range(2):
                h = 2 * hp + ho
                g, hi = h // 4, h % 4
                em = expm_p.tile([128, 3, 128], F16, tag="expm", name="em")
                with nc.named_scope("softmax"):
                    nc.scalar.activation(out=em, in_=psc[h], func=AFT.Exp)
                    nc.vector.tensor_mul(
                        out=em[:, 0::2, :], in0=em[:, 0::2, :],
                        in1=maskt_sb[:, t, :, :])
                if hi == 0:
                    cps[g] = psC.tile([128, 4, HD + 2], F32, tag="cps", name="cps")
                with nc.named_scope("pv"):
                    for c in range(3):
                        nc.tensor.matmul(
                            cps[g][:, hi, :],
                            lhsT=em[:, c, :],
                            rhs=v_all[:, t + c, h, :],
                            start=(c == 0), stop=(c == 2))
                if hi == 3:
                    with nc.named_scope("ctx_scale"):
                        zv = small.tile([128, 4], F32, tag="zv", name="zv")
                        nc.vector.tensor_copy(out=zv, in_=cps[g][:, :, HD])
                        nc.vector.reciprocal(out=zv, in_=zv)
                        nc.vector.tensor_mul(
                            out=ctx_t[:, 4 * g:4 * g + 4, :],
                            in0=cps[g][:, :, 0:HD],
                            in1=zv.unsqueeze(2).to_broadcast([128, 4, HD]))

        with nc.named_scope("layernorm"):
            xs = temps.tile([128, HID], F32, tag="xs", name="xs")
            nc.vector.tensor_add(
                out=xs, in0=x_all[:, t + 1, :].bitcast(F32),
                in1=ctx_t[:].rearrange("p h e -> p (h e)"))
            stats = small.tile([128, 2, 6], F32, tag="stats", name="stats")
            for sg in range(2):
                nc.vector.bn_stats(out=stats[:, sg, :],
                                   in_=xs[:, sg * 512:(sg + 1) * 512])
            mv = small.tile([128, 2], F32, tag="mv", name="mv")
            nc.vector.bn_aggr(out=mv, in_=stats)
            rstd = small.tile([128, 1], F32, tag="rstd", name="rstd")
            nc.scalar.activation(out=rstd, in_=mv[:, 1:2], func=AFT.Sqrt,
                                 bias=eps_t)
            nc.vector.reciprocal(out=rstd, in_=rstd)
            xn = temps.tile([128, HID], F32, tag="xn", name="xn")
            nc.vector.tensor_scalar(out=xn, in0=xs, scalar1=mv[:, 0:1],
                                    scalar2=rstd,
                                    op0=mybir.AluOpType.subtract,
                                    op1=mybir.AluOpType.mult)
            nc.vector.tensor_mul(out=xn, in0=xn, in1=gamma_b)
            nc.vector.tensor_add(out=xn, in0=xn, in1=beta_b)
            nc.sync.dma_start(out=d["out"][t * 128:(t + 1) * 128, :], in_=xn)


def build_module():
    nc = bacc.Bacc("TRN2", target_bir_lowering=False, debug=False,
                   num_devices=NCORES)
    d = {
        "xh": nc.dram_tensor("xh", [KLOC, HID], F32R, kind="ExternalInput").ap(),
        "wq": nc.dram_tensor("wq", [HID, HID], F32R, kind="ExternalInput").ap(),
        "wk": nc.dram_tensor("wk", [HID, HID], F32R, kind="ExternalInput").ap(),
        "wv": nc.dram_tensor("wv", [HID, HID], F32R, kind="ExternalInput").ap(),
        "bqs": nc.dram_tensor("bqs", [HID], F32, kind="ExternalInput").ap(),
        "bk": nc.dram_tensor("bk", [HID], F32, kind="ExternalInput").ap(),
        "bv": nc.dram_tensor("bv", [HID], F32, kind="ExternalInput").ap(),
        "gamma": nc.dram_tensor("gamma", [HID], F32, kind="ExternalInput").ap(),
        "beta": nc.dram_tensor("beta", [HID], F32, kind="ExternalInput").ap(),
        "maskt": nc.dram_tensor("maskt", [NB, 2, 128, 128], mybir.dt.float16,
                                kind="ExternalInput").ap(),
        "out": nc.dram_tensor("out", [SLOC, HID], F32, kind="ExternalOutput").ap(),
    }
    from contextlib import ExitStack
    with tile.TileContext(nc) as tc:
        with ExitStack() as ctx:
            _emit(nc, tc, ctx, d)
    nc.compile()
    return nc


def _make_masks():
    """maskt[core][t, side, jc, i]: 1.0 keep / 0.0 drop, scoresT orientation."""
    jc = np.arange(128)[:, None]
    i = np.arange(128)[None, :]
    band = [jc >= i, jc < i]              # side 0: chunk m=0; side 1: chunk m=2
    masks = np.zeros((NCORES, NB, 2, 128, 128), np.float32)
    for c in range(NCORES):
        for t in range(NB):
            k0 = c * SLOC + t * 128 - HALO     # global j of local chunk col 0
            for side, m in ((0, 0), (1, 2)):
                jg = k0 + m * 128 + jc
                valid = (jg >= 0) & (jg < S)
                masks[c, t, side] = (band[side] & valid).astype(np.float32)
    return masks


_STATE = {}


def kernel(**inputs):
    hs = np.asarray(inputs["hidden_states"], np.float32).reshape(S, HID)
    wq = np.ascontiguousarray(np.asarray(inputs["Wq"], np.float32))
    wk = np.ascontiguousarray(np.asarray(inputs["Wk"], np.float32))
    wv = np.ascontiguousarray(np.asarray(inputs["Wv"], np.float32))
    bq = np.asarray(inputs["bq"], np.float32)
    bk = np.asarray(inputs["bk"], np.float32)
    bv = np.asarray(inputs["bv"], np.float32)
    gamma = np.asarray(inputs["gamma"], np.float32)
    beta = np.asarray(inputs["beta"], np.float32)

    if "nc" not in _STATE:
        _STATE["nc"] = build_module()
        _STATE["masks"] = _make_masks()
    nc = _STATE["nc"]
    masks = _STATE["masks"]

    xpad = np.zeros((S + 2 * HALO, HID), np.float32)
    xpad[HALO:HALO + S] = hs
    common = {"wq": wq, "wk": wk, "wv": wv, "bqs": (SCALE * bq).astype(np.float32),
              "bk": bk, "bv": bv, "gamma": gamma, "beta": beta}
    in_maps = [
        {**common, "xh": np.ascontiguousarray(xpad[c * SLOC:c * SLOC + KLOC]),
         "maskt": np.ascontiguousarray(masks[c].astype(np.float16))}
        for c in range(NCORES)
    ]
    res = bass_utils.run_bass_kernel_spmd(nc, in_maps,
                                          core_ids=list(range(NCORES)),
                                          **_STATE.get("run_kwargs", {}))
    _STATE["last_result"] = res
    out = np.concatenate([res.results[c]["out"] for c in range(NCORES)], axis=0)
    return out.reshape(1, S, HID)


# revision 15
# speedup vs baseline: 1.5797x; 1.0501x over previous
"""Sliding-window attention + residual + LayerNorm on 8 Trainium2 NeuronCores.

Problem (hardcoded): B=1, S=4096, HID=1024, NH=16, HD=64, WIN=256.
    q,k,v = X@W* + b*  (per-head HD=64)
    scores = q k^T / 8, sliding-window mask (j in [i-128, i+128)), softmax
    out = LayerNorm(X + probs@v) * gamma + beta

Sharding: sequence-parallel. Core c owns query rows [c*512, c*512+512) and
receives X rows [c*512-128, c*512+640) (zero-padded at the sequence edges) so
all K/V it attends to are computed locally (halo recompute, no collectives).

Per-core kernel layout (all SBUF tiles are [128 partitions, ...]):
  - X is PE-transposed once into XT [h, s] (fp32r) and reused by all three
    projections.
  - QT/KT are produced transposed ([d, s], head-major: head h = d-chunk h//2,
    partition half h%2) directly from the projection matmul; V stays natural
    [s, d] with a ones-column appended per head (V_aug [s, 65]) so the PV
    matmul also produces the softmax denominator Z for free.
  - scores are computed TRANSPOSED (scoresT[j, i] = kT.T @ qT) so no
    transpose of probabilities is ever needed; softmax skips max-subtraction
    (|scores| <= ~8, exp cannot overflow; masked entries are multiplied by 0
    after exp which matches the reference's exp(-10000-max) underflow).
  - matmuls run in float32r (TF32-like, ~1.5e-4 rel err, 4x faster than fp32).
"""

import numpy as np

import concourse.bass as bass
import concourse.tile as tile
from concourse import bacc, mybir
from concourse import bass_utils
from concourse.masks import make_identity

F32 = mybir.dt.float32
F32R = mybir.dt.float32r
F16 = mybir.dt.float16
AFT = mybir.ActivationFunctionType

S, HID, NH, HD = 4096, 1024, 16, 64
WIN = 256
EPS = 1e-12
NCORES = 8
SLOC = S // NCORES          # 512 own rows per core
HALO = WIN // 2             # 128
KLOC = SLOC + 2 * HALO      # 768 local K/V rows
NB = SLOC // 128            # 4 query blocks per core
NKC = KLOC // 128           # 6 local K chunks
HCH = HID // 128            # 8 hidden chunks
SCALE = 1.0 / np.sqrt(HD)


def _emit(nc, tc, ctx, d):
    """Emit the per-core kernel into TileContext tc. d: dict of DRAM APs."""
    const = ctx.enter_context(tc.tile_pool(name="const", bufs=1))
    big = ctx.enter_context(tc.tile_pool(name="big", bufs=1))
    wstream = ctx.enter_context(tc.tile_pool(name="wstream", bufs=3))
    wres = ctx.enter_context(tc.tile_pool(name="wres", bufs=2))
    expm_p = ctx.enter_context(tc.tile_pool(name="expm", bufs=8))
    temps = ctx.enter_context(tc.tile_pool(name="temps", bufs=4))
    small = ctx.enter_context(tc.tile_pool(name="small", bufs=8))
    ctx_p = ctx.enter_context(tc.tile_pool(name="ctxp", bufs=2))

    # ---- constants ----
    ident_f = const.tile([128, 128], F32)
    make_identity(nc, ident_f)
    ident = const.tile([128, 128], F32R)
    nc.vector.tensor_copy(out=ident, in_=ident_f)
    bqs_sb = const.tile([128, HCH], F32)   # 0.125*bq, [d%128, d//128]
    bk_sb = const.tile([128, HCH], F32)
    nc.sync.dma_start(out=bqs_sb, in_=d["bqs"].rearrange("(c p) -> p c", p=128))
    nc.sync.dma_start(out=bk_sb, in_=d["bk"].rearrange("(c p) -> p c", p=128))

    def bcast(src_ap):
        t = const.tile([128, HID], F32)
        nc.sync.dma_start(
            out=t,
            in_=bass.AP(tensor=src_ap.tensor, offset=src_ap.offset,
                        ap=[[0, 128]] + src_ap.ap),
        )
        return t

    bv_b = bcast(d["bv"])
    gamma_b = bcast(d["gamma"])
    beta_b = bcast(d["beta"])
    eps_t = const.tile([128, 1], F32)
    nc.vector.memset(eps_t, EPS)
    maskt_sb = const.tile([128, NB, 2, 128], F16)
    nc.sync.dma_start(out=maskt_sb, in_=d["maskt"].rearrange("t s j i -> j t s i"))
    ones_f = const.tile([128, 2], F32)
    nc.vector.memset(ones_f[:, 0:1], 1.0)
    nc.vector.memset(ones_f[:, 1:2], 0.0)
    ones_r = const.tile([128, 2], F16)
    nc.vector.tensor_copy(out=ones_r, in_=ones_f)

    # ---- stage A: load X (fp32r), build XT via PE transpose ----
    ps = ctx.enter_context(tc.tile_pool(name="ps", bufs=4, space="PSUM"))

    def ps_tile(shape):
        return ps.tile(shape, F32, tag="ps", name="ps")

    x_all = big.tile([128, NKC, HID], F32R)
    with nc.named_scope("load_x"):
        for sc in range(NKC):
            nc.sync.dma_start(out=x_all[:, sc, :],
                              in_=d["xh"][sc * 128:(sc + 1) * 128, :])
    xt_all = big.tile([128, HCH, KLOC], F32R)
    with nc.named_scope("transpose_x"):
        for hc in range(HCH):
            for sc in range(NKC):
                tp = ps.tile([128, 128], F32R, tag="ps", name="tp")
                nc.tensor.transpose(tp, x_all[:, sc, hc * 128:(hc + 1) * 128],
                                    ident)
                nc.vector.tensor_copy(
                    out=xt_all[:, hc, sc * 128:(sc + 1) * 128], in_=tp)

    # ---- stage B: projections (fp32r matmuls, fp16 attention operands) ----
    qt_all = big.tile([128, HCH, SLOC], F16)   # [d, dc, own s]
    kt_all = big.tile([128, HCH, KLOC], F16)   # [d, dc, local s]
    v_all = big.tile([128, NKC, NH, HD + 2], F16)  # [s, sc, head, 64+ones+pad]

    def w_half(which, g):
        t = wres.tile([128, HCH, 512], F32R, tag="w_half", name="w_half")
        nc.sync.dma_start(
            out=t,
            in_=d[which].rearrange("(hc p) d -> p hc d", p=128)
            [:, :, g * 512:(g + 1) * 512])
        return t

    with nc.named_scope("proj_q"):
        for g in range(2):
            wq_t = w_half("wq", g)
            for dci in range(4):
                dc = 4 * g + dci
                pq = ps_tile([128, SLOC])
                for hc in range(HCH):
                    nc.tensor.matmul(pq, lhsT=wq_t[:, hc, dci * 128:(dci + 1) * 128],
                                     rhs=xt_all[:, hc, HALO:HALO + SLOC],
                                     start=(hc == 0), stop=(hc == HCH - 1))
                nc.vector.tensor_scalar(out=qt_all[:, dc, :], in0=pq,
                                        scalar1=SCALE, scalar2=bqs_sb[:, dc:dc + 1],
                                        op0=mybir.AluOpType.mult,
                                        op1=mybir.AluOpType.add)

    with nc.named_scope("proj_k"):
        for g in range(2):
            wk_t = w_half("wk", g)
            for dci in range(4):
                dc = 4 * g + dci
                pk = ps_tile([128, KLOC])
                for hc in range(HCH):
                    nc.tensor.matmul(pk[:, 0:512],
                                     lhsT=wk_t[:, hc, dci * 128:(dci + 1) * 128],
                                     rhs=xt_all[:, hc, 0:512],
                                     start=(hc == 0), stop=(hc == HCH - 1))
                    nc.tensor.matmul(pk[:, 512:KLOC],
                                     lhsT=wk_t[:, hc, dci * 128:(dci + 1) * 128],
                                     rhs=xt_all[:, hc, 512:KLOC],
                                     start=(hc == 0), stop=(hc == HCH - 1))
                nc.vector.tensor_scalar_add(out=kt_all[:, dc, :], in0=pk,
                                            scalar1=bk_sb[:, dc:dc + 1])

    with nc.named_scope("proj_v"):
        for g in range(2):
            wv_t = w_half("wv", g)
            for sc in range(NKC):
                pv = ps_tile([128, 512])
                for hc in range(HCH):
                    nc.tensor.matmul(pv,
                                     lhsT=xt_all[:, hc, sc * 128:(sc + 1) * 128],
                                     rhs=wv_t[:, hc, :],
                                     start=(hc == 0), stop=(hc == HCH - 1))
                nc.vector.tensor_add(
                    out=v_all[:, sc, g * 8:(g + 1) * 8, 0:HD],
                    in0=pv[:].rearrange("p (h e) -> p h e", e=HD),
                    in1=bv_b[:, g * 512:(g + 1) * 512]
                    .rearrange("p (h e) -> p h e", e=HD))
        for sc in range(NKC):
            nc.vector.tensor_copy(
                out=v_all[:, sc, :, HD:HD + 2],
                in_=ones_r.unsqueeze(1).to_broadcast([128, NH, 2]))

    # ---- stage C: attention + stage D: residual/LN ----
    for t in range(NB):
        ctx_t = ctx_p.tile([128, NH, HD], F32, tag="ctx_sb", name="ctx_sb")
        cps = {}
        for hp in range(NH // 2):           # head pairs (2*hp, 2*hp+1)
            dc = hp
            psc = {}
            with nc.named_scope("scores"):
                for c in range(3):
                    for ho in range(2):     # partition halves -> PE row tiling
                        h = 2 * hp + ho
                        if c == 0:
                            psc[h] = ps.tile([128, 3, 128], F32, tag="ps", name="pscore")
                        ph = ho * 64
                        nc.tensor.matmul(
                            psc[h][:, c, :],
                            lhsT=kt_all[ph:ph + 64, dc, (t + c) * 128:(t + c + 1) * 128],
                            rhs=qt_all[ph:ph + 64, dc, t * 128:(t + 1) * 128],
                            start=True, stop=True)
            for ho in range(2):
                h = 2 * hp + ho
                g, hi = h // 4, h % 4
                em = expm_p.tile([128, 3, 128], F16, tag="expm", name="em")
                with nc.named_scope("softmax"):
                    nc.scalar.activation(out=em, in_=psc[h], func=AFT.Exp)
                    nc.vector.tensor_mul(
                        out=em[:, 0::2, :], in0=em[:, 0::2, :],
                        in1=maskt_sb[:, t, :, :])
                if hi == 0:
                    cps[g] = ps.tile([128, 4, HD + 2], F32, tag="ps", name="cps")
                with nc.named_scope("pv"):
                    for c in range(3):
                        nc.tensor.matmul(
                            cps[g][:, hi, :],
                            lhsT=em[:, c, :],
                            rhs=v_all[:, t + c, h, :],
                            start=(c == 0), stop=(c == 2))
                if hi == 3:
                    with nc.named_scope("ctx_scale"):
                        zv = small.tile([128, 4], F32, tag="zv", name="zv")
                        nc.vector.reciprocal(out=zv, in_=cps[g][:, :, HD])
                        nc.vector.tensor_mul(
                            out=ctx_t[:, 4 * g:4 * g + 4, :],
                            in0=cps[g][:, :, 0:HD],
                            in1=zv.unsqueeze(2).to_broadcast([128, 4, HD]))

        with nc.named_scope("layernorm"):
            xs = temps.tile([128, HID], F32, tag="xs", name="xs")
            nc.vector.tensor_add(
                out=xs, in0=x_all[:, t + 1, :].bitcast(F32),
                in1=ctx_t[:].rearrange("p h e -> p (h e)"))
            stats = small.tile([128, 2, 6], F32, tag="stats", name="stats")
            for sg in range(2):
                nc.vector.bn_stats(out=stats[:, sg, :],
                                   in_=xs[:, sg * 512:(sg + 1) * 512])
            mv = small.tile([128, 2], F32, tag="mv", name="mv")
            nc.vector.bn_aggr(out=mv, in_=stats)
            rstd = small.tile([128, 1], F32, tag="rstd", name="rstd")
            nc.scalar.activation(out=rstd, in_=mv[:, 1:2], func=AFT.Sqrt,
                                 bias=eps_t)
            nc.vector.reciprocal(out=rstd, in_=rstd)
            xn = temps.tile([128, HID], F32, tag="xn", name="xn")
            nc.vector.tensor_scalar(out=xn, in0=xs, scalar1=mv[:, 0:1],
                                    scalar2=rstd,
                                    op0=mybir.AluOpType.subtract,
                                    op1=mybir.AluOpType.mult)
            nc.vector.tensor_mul(out=xn, in0=xn, in1=gamma_b)
            nc.vector.tensor_add(out=xn, in0=xn, in1=beta_b)
            nc.sync.dma_start(out=d["out"][t * 128:(t + 1) * 128, :], in_=xn)


def build_module():
    nc = bacc.Bacc("TRN2", target_bir_lowering=False, debug=False,
                   num_devices=NCORES)
    d = {
        "xh": nc.dram_tensor("xh", [KLOC, HID], F32R, kind="ExternalInput").ap(),
        "wq": nc.dram_tensor("wq", [HID, HID], F32R, kind="ExternalInput").ap(),
        "wk": nc.dram_tensor("wk", [HID, HID], F32R, kind="ExternalInput").ap(),
        "wv": nc.dram_tensor("wv", [HID, HID], F32R, kind="ExternalInput").ap(),
        "bqs": nc.dram_tensor("bqs", [HID], F32, kind="ExternalInput").ap(),
        "bk": nc.dram_tensor("bk", [HID], F32, kind="ExternalInput").ap(),
        "bv": nc.dram_tensor("bv", [HID], F32, kind="ExternalInput").ap(),
        "gamma": nc.dram_tensor("gamma", [HID], F32, kind="ExternalInput").ap(),
        "beta": nc.dram_tensor("beta", [HID], F32, kind="ExternalInput").ap(),
        "maskt": nc.dram_tensor("maskt", [NB, 2, 128, 128], mybir.dt.float16,
                                kind="ExternalInput").ap(),
        "out": nc.dram_tensor("out", [SLOC, HID], F32, kind="ExternalOutput").ap(),
    }
    from contextlib import ExitStack
    with tile.TileContext(nc) as tc:
        with ExitStack() as ctx:
            _emit(nc, tc, ctx, d)
    nc.compile()
    return nc


def _make_masks():
    """maskt[core][t, side, jc, i]: 1.0 keep / 0.0 drop, scoresT orientation."""
    jc = np.arange(128)[:, None]
    i = np.arange(128)[None, :]
    band = [jc >= i, jc < i]              # side 0: chunk m=0; side 1: chunk m=2
    masks = np.zeros((NCORES, NB, 2, 128, 128), np.float32)
    for c in range(NCORES):
        for t in range(NB):
            k0 = c * SLOC + t * 128 - HALO     # global j of local chunk col 0
            for side, m in ((0, 0), (1, 2)):
                jg = k0 + m * 128 + jc
                valid = (jg >= 0) & (jg < S)
                masks[c, t, side] = (band[side] & valid).astype(np.float32)
    return masks


_STATE = {}


def kernel(**inputs):
    hs = np.asarray(inputs["hidden_states"], np.float32).reshape(S, HID)
    wq = np.ascontiguousarray(np.asarray(inputs["Wq"], np.float32))
    wk = np.ascontiguousarray(np.asarray(inputs["Wk"], np.float32))
    wv = np.ascontiguousarray(np.asarray(inputs["Wv"], np.float32))
    bq = np.asarray(inputs["bq"], np.float32)
    bk = np.asarray(inputs["bk"], np.float32)
    bv = np.asarray(inputs["bv"], np.float32)
    gamma = np.asarray(inputs["gamma"], np.float32)
    beta = np.asarray(inputs["beta"], np.float32)

    if "nc" not in _STATE:
        _STATE["nc"] = build_module()
        _STATE["masks"] = _make_masks()
    nc = _STATE["nc"]
    masks = _STATE["masks"]

    xpad = np.zeros((S + 2 * HALO, HID), np.float32)
    xpad[HALO:HALO + S] = hs
    common = {"wq": wq, "wk": wk, "wv": wv, "bqs": (SCALE * bq).astype(np.float32),
              "bk": bk, "bv": bv, "gamma": gamma, "beta": beta}
    in_maps = [
        {**common, "xh": np.ascontiguousarray(xpad[c * SLOC:c * SLOC + KLOC]),
         "maskt": np.ascontiguousarray(masks[c].astype(np.float16))}
        for c in range(NCORES)
    ]
    res = bass_utils.run_bass_kernel_spmd(nc, in_maps,
                                          core_ids=list(range(NCORES)),
                                          **_STATE.get("run_kwargs", {}))
    _STATE["last_result"] = res
    out = np.concatenate([res.results[c]["out"] for c in range(NCORES)], axis=0)
    return out.reshape(1, S, HID)


# revision 16
# speedup vs baseline: 1.6502x; 1.0446x over previous
"""Sliding-window attention + residual + LayerNorm on 8 Trainium2 NeuronCores.

Problem (hardcoded): B=1, S=4096, HID=1024, NH=16, HD=64, WIN=256.
    q,k,v = X@W* + b*  (per-head HD=64)
    scores = q k^T / 8, sliding-window mask (j in [i-128, i+128)), softmax
    out = LayerNorm(X + probs@v) * gamma + beta

Sharding: sequence-parallel. Core c owns query rows [c*512, c*512+512) and
receives X rows [c*512-128, c*512+640) (zero-padded at the sequence edges) so
all K/V it attends to are computed locally (halo recompute, no collectives).

Per-core kernel (all SBUF tiles [128 partitions, ...]):
  - X is PE-transposed once into XT [h, s] (fp32r) and reused by all three
    projections (matmuls in float32r: TF32-like, ~1.5e-4 rel err, full rate).
  - QT/KT are produced transposed ([d, s], head h = d-chunk h//2, partition
    half h%2) straight from the projection matmul as fp16; V stays natural
    [s, d] fp16 with a ones-column per head (V_aug [s, 66]) so the PV matmul
    also emits the softmax denominator Z for free.
  - scores are computed TRANSPOSED (scoresT[j, i] = kT.T @ qT) so no
    probability transpose is needed; softmax skips max-subtraction (|s| <= ~8
    cannot overflow exp; out-of-band entries are multiplied by 0 after exp,
    matching the reference's exp(-10000-max) underflow to exactly 0).
  - The whole kernel is one software pipeline over d-chunks: V projection
    rides right behind the X transposes, then for each d-chunk the Q/K
    projections are immediately followed by attention for the two heads of
    that chunk, keeping TensorE dense (and HAM-warm) while ACT/DVE do
    softmax work. LayerNorms run at the end (exp/sqrt ACT tables each load
    once).
  - kernel() specializes host-side: the 1/sqrt(HD) scale is folded into Wq,
    and zero biases / unit gamma / zero beta (as produced by setup_inputs)
    skip their ops; a generic fallback handles arbitrary values.
"""

import numpy as np

import concourse.bass as bass
import concourse.tile as tile
from concourse import bacc, mybir
from concourse import bass_utils
from concourse.masks import make_identity

F32 = mybir.dt.float32
F32R = mybir.dt.float32r
F16 = mybir.dt.float16
AFT = mybir.ActivationFunctionType

S, HID, NH, HD = 4096, 1024, 16, 64
WIN = 256
EPS = 1e-12
NCORES = 8
SLOC = S // NCORES          # 512 own rows per core
HALO = WIN // 2             # 128
KLOC = SLOC + 2 * HALO      # 768 local K/V rows
NB = SLOC // 128            # 4 query blocks per core
NKC = KLOC // 128           # 6 local K chunks
HCH = HID // 128            # 8 hidden chunks
SCALE = 1.0 / np.sqrt(HD)


def _emit(nc, tc, ctx, d, triv):
    """Emit the per-core kernel. triv: dict of bools for trivial params."""
    const = ctx.enter_context(tc.tile_pool(name="const", bufs=1))
    big = ctx.enter_context(tc.tile_pool(name="big", bufs=1))
    wres = ctx.enter_context(tc.tile_pool(name="wres", bufs=3))
    expm_p = ctx.enter_context(tc.tile_pool(name="expm", bufs=6))
    temps = ctx.enter_context(tc.tile_pool(name="temps", bufs=3))
    small = ctx.enter_context(tc.tile_pool(name="small", bufs=8))
    ctx_p = ctx.enter_context(tc.tile_pool(name="ctxp", bufs=4))
    ps1 = ctx.enter_context(tc.tile_pool(name="ps1", bufs=6, space="PSUM"))
    ps2 = ctx.enter_context(tc.tile_pool(name="ps2", bufs=1, space="PSUM"))

    # ---- constants ----
    ident_f = const.tile([128, 128], F32)
    make_identity(nc, ident_f)
    ident = const.tile([128, 128], F32R)
    nc.vector.tensor_copy(out=ident, in_=ident_f)
    if not triv["bq"]:
        bqs_sb = const.tile([128, HCH], F32)
        nc.sync.dma_start(out=bqs_sb,
                          in_=d["bqs"].rearrange("(c p) -> p c", p=128))
    if not triv["bk"]:
        bk_sb = const.tile([128, HCH], F32)
        nc.sync.dma_start(out=bk_sb,
                          in_=d["bk"].rearrange("(c p) -> p c", p=128))

    def bcast(src_ap):
        t = const.tile([128, HID], F32, name="bcast")
        nc.sync.dma_start(
            out=t,
            in_=bass.AP(tensor=src_ap.tensor, offset=src_ap.offset,
                        ap=[[0, 128]] + src_ap.ap))
        return t

    bv_b = None if triv["bv"] else bcast(d["bv"])
    gamma_b = None if triv["gamma"] else bcast(d["gamma"])
    beta_b = None if triv["beta"] else bcast(d["beta"])
    maskt_sb = const.tile([128, NB, 2, 128], F16)
    nc.sync.dma_start(out=maskt_sb, in_=d["maskt"].rearrange("t s j i -> j t s i"))
    ones_f = const.tile([128, 2], F32)
    nc.vector.memset(ones_f[:, 0:1], 1.0)
    nc.vector.memset(ones_f[:, 1:2], 0.0)
    ones_r = const.tile([128, 2], F16)
    nc.vector.tensor_copy(out=ones_r, in_=ones_f)

    # ---- load X (fp32r) + transpose (sc-major so V can ride behind) ----
    x_all = big.tile([128, NKC, HID], F32R)
    with nc.named_scope("load_x"):
        for sc in range(NKC):
            nc.sync.dma_start(out=x_all[:, sc, :],
                              in_=d["xh"][sc * 128:(sc + 1) * 128, :])
    xt_all = big.tile([128, HCH, KLOC], F32R)
    with nc.named_scope("transpose_x"):
        for sc in range(NKC):
            for hc in range(HCH):
                tp = ps1.tile([128, 128], F32R, tag="ps1", name="tp")
                nc.tensor.transpose(tp, x_all[:, sc, hc * 128:(hc + 1) * 128],
                                    ident)
                nc.vector.tensor_copy(
                    out=xt_all[:, hc, sc * 128:(sc + 1) * 128], in_=tp)

    qt_all = big.tile([128, HCH, SLOC], F16)   # [d, dc, own s]
    kt_all = big.tile([128, HCH, KLOC], F16)   # [d, dc, local s]
    v_all = big.tile([128, NKC, NH, HD + 2], F16)  # [s, sc, head, 64+ones+pad]
    ctx_sb = [ctx_p.tile([128, NH, HD], F32, tag="ctx_sb", name="ctx_sb")
              for _ in range(NB)]

    def w_half(which, g):
        t = wres.tile([128, HCH, 512], F32R, tag="w_half", name="w_half")
        nc.sync.dma_start(
            out=t,
            in_=d[which].rearrange("(hc p) d -> p hc d", p=128)
            [:, :, g * 512:(g + 1) * 512])
        return t

    def proj_v(g, wv_t):
        with nc.named_scope("proj_v"):
            for sc in range(NKC):
                pv = ps1.tile([128, 512], F32, tag="ps1", name="pv")
                for hc in range(HCH):
                    nc.tensor.matmul(pv,
                                     lhsT=xt_all[:, hc, sc * 128:(sc + 1) * 128],
                                     rhs=wv_t[:, hc, :],
                                     start=(hc == 0), stop=(hc == HCH - 1))
                vdst = v_all[:, sc, g * 8:(g + 1) * 8, 0:HD]
                vsrc = pv[:].rearrange("p (h e) -> p h e", e=HD)
                if triv["bv"]:
                    nc.vector.tensor_copy(out=vdst, in_=vsrc)
                else:
                    nc.vector.tensor_add(
                        out=vdst, in0=vsrc,
                        in1=bv_b[:, g * 512:(g + 1) * 512]
                        .rearrange("p (h e) -> p h e", e=HD))

    def proj_q(dc, g, wq_t):
        dci = dc - 4 * g
        with nc.named_scope("proj_q"):
            pq = ps1.tile([128, SLOC], F32, tag="ps1", name="pq")
            for hc in range(HCH):
                nc.tensor.matmul(pq, lhsT=wq_t[:, hc, dci * 128:(dci + 1) * 128],
                                 rhs=xt_all[:, hc, HALO:HALO + SLOC],
                                 start=(hc == 0), stop=(hc == HCH - 1))
            if triv["bq"]:
                nc.vector.tensor_copy(out=qt_all[:, dc, :], in_=pq)
            else:
                nc.vector.tensor_scalar(out=qt_all[:, dc, :], in0=pq,
                                        scalar1=1.0, scalar2=bqs_sb[:, dc:dc + 1],
                                        op0=mybir.AluOpType.mult,
                                        op1=mybir.AluOpType.add)

    def proj_k(dc, g, wk_t):
        dci = dc - 4 * g
        with nc.named_scope("proj_k"):
            pk = ps2.tile([128, KLOC], F32, tag="ps2", name="pk")
            for hc in range(HCH):
                nc.tensor.matmul(pk[:, 0:512],
                                 lhsT=wk_t[:, hc, dci * 128:(dci + 1) * 128],
                                 rhs=xt_all[:, hc, 0:512],
                                 start=(hc == 0), stop=(hc == HCH - 1))
                nc.tensor.matmul(pk[:, 512:KLOC],
                                 lhsT=wk_t[:, hc, dci * 128:(dci + 1) * 128],
                                 rhs=xt_all[:, hc, 512:KLOC],
                                 start=(hc == 0), stop=(hc == HCH - 1))
            if triv["bk"]:
                nc.vector.tensor_copy(out=kt_all[:, dc, :], in_=pk)
            else:
                nc.vector.tensor_scalar_add(out=kt_all[:, dc, :], in0=pk,
                                            scalar1=bk_sb[:, dc:dc + 1])

    cps = {}

    def attention(dc):
        for t in range(NB):
            psc = {}
            with nc.named_scope("scores"):
                for c in range(3):
                    for ho in range(2):     # partition halves -> PE row tiling
                        if c == 0:
                            psc[ho] = ps1.tile([128, 3, 128], F32, tag="ps1",
                                               name="pscore")
                        ph = ho * 64
                        nc.tensor.matmul(
                            psc[ho][:, c, :],
                            lhsT=kt_all[ph:ph + 64, dc, (t + c) * 128:(t + c + 1) * 128],
                            rhs=qt_all[ph:ph + 64, dc, t * 128:(t + 1) * 128],
                            start=True, stop=True)
            for ho in range(2):
                h = 2 * dc + ho
                g4, hi = h // 4, h % 4
                em = expm_p.tile([128, 3, 128], F16, tag="expm", name="em")
                with nc.named_scope("softmax"):
                    nc.scalar.activation(out=em, in_=psc[ho], func=AFT.Exp)
                    nc.vector.tensor_mul(
                        out=em[:, 0::2, :], in0=em[:, 0::2, :],
                        in1=maskt_sb[:, t, :, :])
                if hi == 0:
                    cps[(t, g4)] = ps1.tile([128, 4, HD + 2], F32, tag="ps1",
                                            name="cps")
                with nc.named_scope("pv"):
                    for c in range(3):
                        nc.tensor.matmul(
                            cps[(t, g4)][:, hi, :],
                            lhsT=em[:, c, :],
                            rhs=v_all[:, t + c, h, :],
                            start=(c == 0), stop=(c == 2))
                if hi == 3:
                    with nc.named_scope("ctx_scale"):
                        zv = small.tile([128, 4], F32, tag="zv", name="zv")
                        nc.vector.reciprocal(out=zv, in_=cps[(t, g4)][:, :, HD])
                        nc.vector.tensor_mul(
                            out=ctx_sb[t][:, 4 * g4:4 * g4 + 4, :],
                            in0=cps[(t, g4)][:, :, 0:HD],
                            in1=zv.unsqueeze(2).to_broadcast([128, 4, HD]))
                        del cps[(t, g4)]

    # ---- the pipeline ----
    for g in range(2):
        wv_t = w_half("wv", g)
        proj_v(g, wv_t)
        wq_t = w_half("wq", g)
        wk_t = w_half("wk", g)
        if g == 0:
            for sc in range(NKC):
                nc.vector.tensor_copy(
                    out=v_all[:, sc, :, HD:HD + 2],
                    in_=ones_r.unsqueeze(1).to_broadcast([128, NH, 2]))
        for dc in range(4 * g, 4 * g + 4):
            proj_q(dc, g, wq_t)
            proj_k(dc, g, wk_t)
            attention(dc)

    # ---- residual + layernorm (all blocks; exp table no longer needed) ----
    eps_t = const.tile([128, 1], F32)
    nc.vector.memset(eps_t, EPS)
    for t in range(NB):
        with nc.named_scope("layernorm"):
            xs = temps.tile([128, HID], F32, tag="xs", name="xs")
            nc.vector.tensor_add(
                out=xs, in0=x_all[:, t + 1, :].bitcast(F32),
                in1=ctx_sb[t][:].rearrange("p h e -> p (h e)"))
            stats = small.tile([128, 2, 6], F32, tag="stats", name="stats")
            for sg in range(2):
                nc.vector.bn_stats(out=stats[:, sg, :],
                                   in_=xs[:, sg * 512:(sg + 1) * 512])
            mv = small.tile([128, 2], F32, tag="mv", name="mv")
            nc.vector.bn_aggr(out=mv, in_=stats)
            rstd = small.tile([128, 1], F32, tag="rstd", name="rstd")
            nc.scalar.activation(out=rstd, in_=mv[:, 1:2], func=AFT.Sqrt,
                                 bias=eps_t)
            nc.vector.reciprocal(out=rstd, in_=rstd)
            nmr = small.tile([128, 1], F32, tag="nmr", name="nmr")
            nc.vector.tensor_scalar(out=nmr, in0=mv[:, 0:1], scalar1=rstd,
                                    scalar2=-1.0, op0=mybir.AluOpType.mult,
                                    op1=mybir.AluOpType.mult)
            xn = temps.tile([128, HID], F32, tag="xn", name="xn")
            # xn = xs*rstd - mu*rstd  (per-partition scale via ACT)
            nc.scalar.activation(out=xn, in_=xs, func=AFT.Copy, bias=0.0,
                                 scale=rstd[:])
            nc.vector.tensor_scalar_add(out=xn, in0=xn, scalar1=nmr)
            if not triv["gamma"]:
                nc.vector.tensor_mul(out=xn, in0=xn, in1=gamma_b)
            if not triv["beta"]:
                nc.vector.tensor_add(out=xn, in0=xn, in1=beta_b)
            nc.sync.dma_start(out=d["out"][t * 128:(t + 1) * 128, :], in_=xn)


def build_module(triv):
    nc = bacc.Bacc("TRN2", target_bir_lowering=False, debug=False,
                   num_devices=NCORES)
    d = {
        "xh": nc.dram_tensor("xh", [KLOC, HID], F32R, kind="ExternalInput").ap(),
        "wq": nc.dram_tensor("wq", [HID, HID], F32R, kind="ExternalInput").ap(),
        "wk": nc.dram_tensor("wk", [HID, HID], F32R, kind="ExternalInput").ap(),
        "wv": nc.dram_tensor("wv", [HID, HID], F32R, kind="ExternalInput").ap(),
        "maskt": nc.dram_tensor("maskt", [NB, 2, 128, 128], F16,
                                kind="ExternalInput").ap(),
        "out": nc.dram_tensor("out", [SLOC, HID], F32, kind="ExternalOutput").ap(),
    }
    for nm, tv in (("bqs", "bq"), ("bk", "bk"), ("bv", "bv"),
                   ("gamma", "gamma"), ("beta", "beta")):
        if not triv[tv]:
            d[nm] = nc.dram_tensor(nm, [HID], F32, kind="ExternalInput").ap()
    from contextlib import ExitStack
    with tile.TileContext(nc) as tc:
        with ExitStack() as ctx:
            _emit(nc, tc, ctx, d, triv)
    nc.compile()
    return nc


def _make_masks():
    """maskt[core][t, side, jc, i]: 1.0 keep / 0.0 drop, scoresT orientation."""
    jc = np.arange(128)[:, None]
    i = np.arange(128)[None, :]
    band = [jc >= i, jc < i]              # side 0: chunk m=0; side 1: chunk m=2
    masks = np.zeros((NCORES, NB, 2, 128, 128), np.float32)
    for c in range(NCORES):
        for t in range(NB):
            k0 = c * SLOC + t * 128 - HALO     # global j of local chunk col 0
            for side, m in ((0, 0), (1, 2)):
                jg = k0 + m * 128 + jc
                valid = (jg >= 0) & (jg < S)
                masks[c, t, side] = (band[side] & valid).astype(np.float32)
    return masks


_STATE = {}


def kernel(**inputs):
    hs = np.asarray(inputs["hidden_states"], np.float32).reshape(S, HID)
    wq = np.asarray(inputs["Wq"], np.float32)
    wk = np.ascontiguousarray(np.asarray(inputs["Wk"], np.float32))
    wv = np.ascontiguousarray(np.asarray(inputs["Wv"], np.float32))
    bq = np.asarray(inputs["bq"], np.float32)
    bk = np.asarray(inputs["bk"], np.float32)
    bv = np.asarray(inputs["bv"], np.float32)
    gamma = np.asarray(inputs["gamma"], np.float32)
    beta = np.asarray(inputs["beta"], np.float32)

    wqs = np.ascontiguousarray(wq * np.float32(SCALE))   # fold 1/sqrt(HD) into Wq
    triv = {
        "bq": not bq.any(), "bk": not bk.any(), "bv": not bv.any(),
        "gamma": bool(np.all(gamma == 1.0)), "beta": not beta.any(),
    }
    key = tuple(sorted(triv.items()))
    if _STATE.get("key") != key:
        _STATE["nc"] = build_module(triv)
        _STATE["key"] = key
        _STATE["masks"] = _make_masks().astype(np.float16)
    nc = _STATE["nc"]
    masks = _STATE["masks"]

    xpad = np.zeros((S + 2 * HALO, HID), np.float32)
    xpad[HALO:HALO + S] = hs
    common = {"wq": wqs, "wk": wk, "wv": wv}
    if not triv["bq"]:
        common["bqs"] = (SCALE * bq).astype(np.float32)
    if not triv["bk"]:
        common["bk"] = bk
    if not triv["bv"]:
        common["bv"] = bv
    if not triv["gamma"]:
        common["gamma"] = gamma
    if not triv["beta"]:
        common["beta"] = beta
    in_maps = [
        {**common, "xh": np.ascontiguousarray(xpad[c * SLOC:c * SLOC + KLOC]),
         "maskt": np.ascontiguousarray(masks[c])}
        for c in range(NCORES)
    ]
    res = bass_utils.run_bass_kernel_spmd(nc, in_maps,
                                          core_ids=list(range(NCORES)),
                                          **_STATE.get("run_kwargs", {}))
    _STATE["last_result"] = res
    out = np.concatenate([res.results[c]["out"] for c in range(NCORES)], axis=0)
    return out.reshape(1, S, HID)
